# revision 1
# baseline (speedup 1.0000x reference)
"""BiDirectionalCrossAttention Trainium2 kernel (8-core data parallel).

Math (per sample m, matching the reference):
  q1 = x @ Wq1.T + bq1   (x = protein)     k1,v1 from y (ligand)
  q2 = y @ Wq2.T + bq2                     k2,v2 from x
  S[h,e]   = q[h,:] . k[e,:] / sqrt(64)    (heads mix: 8x8 scores per sample)
  A        = softmax_e(S)
  out[h,:] = sum_e A[h,e] v[e,:]
  protein_out = out1 @ Wo1.T + bo1 ; ligand_out = out2 @ Wo2.T + bo2

Mapping:
  - batch is sharded 8 ways (4096 samples/core); weights replicated.
  - inputs are passed transposed (xT [512, 4096]); the 128-row K-chunks of
    x.T serve directly as matmul stationary operands; biases ride as a 513th
    row combined with a ones-row K=1 matmul.
  - projections + output projections run on the TensorEngine in float32r.
  - per-sample attention runs with samples on partitions, entirely in bf16
    so every DVE tensor_tensor op hits the 2x perf mode:
      * v is evacuated d-major (vT layout [d*8+e]) so the A@V product's
        operands are all innermost-stride-1;
      * exp output (attention weights) is bf16 and pre-normalized by 1/z
        at [128, 64] (cheap) instead of normalizing the [128, 512] output;
      * the QK d-reduction is a full bf16 binary tree (32->1);
      * attention output is produced d-major and the output-projection
        weights are host-permuted to match, so no normalize pass and a
        bf16 TensorE transpose (1 cyc/row).
  - the big products are split DVE/GPSIMD by head slices to balance engines.
  - per-(tile, direction) work is software-pipelined across the batch.
"""

import os

import numpy as np

import concourse.bacc as bacc
import concourse.mybir as mybir
import concourse.tile as tile
from concourse import bass_utils

B, NF = 32768, 512
H, DH = 8, 64
NCORES = 8
BC = B // NCORES          # samples per core
MT = 128                  # attention tile (samples)
ST = 512                  # projection super-tile (samples)
N_ST = BC // ST
N_MT = ST // MT
SCALE = 8.0               # sqrt(DH)

f32 = mybir.dt.float32
f32r = mybir.dt.float32r
bf16 = mybir.dt.bfloat16

WNAMES = ["q1", "k1", "v1", "q2", "k2", "v2"]

_CACHE: dict = {}


def _iget(name, default):
    return int(os.environ.get(name, str(default)))


def _emit(nc, tc, dr):
    from contextlib import ExitStack

    X = mybir.AxisListType.X
    ADD = mybir.AluOpType.add
    EXP = mybir.ActivationFunctionType.Exp
    IDENT_FN = mybir.ActivationFunctionType.Identity

    sc = _iget("BIDIR_SC_SPLIT", 8)    # qk-product heads on DVE (rest GPSIMD)
    hv = _iget("BIDIR_AV_SPLIT", 3)    # av-product h-slices on DVE (rest GPSIMD)
    tr1_gp = _iget("BIDIR_TR1_GP", 0)  # qk tree level-1 heads on GPSIMD
    av1_gp = _iget("BIDIR_AV1_GP", 0)  # av tree level-1 h-slices on GPSIMD
    lag = _iget("BIDIR_LAG", 2)

    with ExitStack() as ctx:
        wpool = ctx.enter_context(tc.tile_pool(name="weights", bufs=1))
        xpool = ctx.enter_context(tc.tile_pool(name="xstage", bufs=1))
        qkv_pool = ctx.enter_context(tc.tile_pool(name="qkv", bufs=_iget("BIDIR_QKV_BUFS", 2)))
        big_pool = ctx.enter_context(tc.tile_pool(name="bigp", bufs=_iget("BIDIR_BIG_BUFS", 2)))
        p2_pool = ctx.enter_context(tc.tile_pool(name="p2p", bufs=_iget("BIDIR_P2_BUFS", 3)))
        prod_pool = ctx.enter_context(tc.tile_pool(name="prod", bufs=_iget("BIDIR_TR_BUFS", 2)))
        small_pool = ctx.enter_context(tc.tile_pool(name="small", bufs=_iget("BIDIR_SMALL_BUFS", 5)))
        ao_pool = ctx.enter_context(tc.tile_pool(name="aoT", bufs=_iget("BIDIR_AO_BUFS", 1)))
        out_pool = ctx.enter_context(tc.tile_pool(name="outb", bufs=_iget("BIDIR_OUT_BUFS", 4)))
        pp = ctx.enter_context(tc.tile_pool(name="pproj", bufs=2, space="PSUM"))
        pt = ctx.enter_context(tc.tile_pool(name="ptrans", bufs=1, space="PSUM"))
        po = ctx.enter_context(tc.tile_pool(name="pout", bufs=1, space="PSUM"))

        # ---- static weights ----
        W = {}
        for n in WNAMES:
            chunks = []
            for c in range(4):
                t = wpool.tile([128, NF], f32r, tag=f"w_{n}_{c}", name=f"w_{n}_{c}")
                nc.sync.dma_start(t[:], dr[f"w_{n}"].ap()[128 * c:128 * (c + 1), :])
                chunks.append(t)
            bt = wpool.tile([1, NF], f32r, tag=f"w_{n}_b", name=f"w_{n}_b")
            nc.sync.dma_start(bt[:], dr[f"w_{n}"].ap()[NF:NF + 1, :])
            W[n] = (chunks, bt)
        WO = {}
        for n in ("o1", "o2"):
            WO[n] = []
            for c in range(4):
                t = wpool.tile([128, NF], f32r, tag=f"wo_{n}_{c}", name=f"wo_{n}_{c}")
                nc.sync.dma_start(t[:], dr[f"w{n}T"].ap()[128 * c:128 * (c + 1), :])
                WO[n].append(t)
        bo_sb = {}
        for n in ("o1", "o2"):
            t = wpool.tile([128, 4], f32, tag=f"bo_{n}", name=f"bo_{n}")
            nc.sync.dma_start(t[:], dr[f"b{n}c"].ap())
            bo_sb[n] = t
        ones = wpool.tile([1, MT], f32r, tag="ones", name="ones")
        nc.sync.dma_start(ones[:], dr["ones_row"].ap())
        ident = wpool.tile([128, 128], bf16, tag="ident", name="ident")
        nc.sync.dma_start(ident[:], dr["ident"].ap())

        def load_supertile(s):
            ssl = slice(ST * s, ST * (s + 1))
            xs, ys = [], []
            for c in range(4):
                xt = xpool.tile([128, ST], f32r, tag=f"xs{c}", name=f"xs{c}")
                nc.sync.dma_start(xt[:], dr["xT"].ap()[128 * c:128 * (c + 1), ssl])
                xs.append(xt)
                yt = xpool.tile([128, ST], f32r, tag=f"ys{c}", name=f"ys{c}")
                nc.sync.dma_start(yt[:], dr["yT"].ap()[128 * c:128 * (c + 1), ssl])
                ys.append(yt)
            aoT = {1: ao_pool.tile([128, 4, ST], f32r, tag="aoT1", name="aoT1"),
                   2: ao_pool.tile([128, 4, ST], f32r, tag="aoT2", name="aoT2")}
            return {"xs": xs, "ys": ys, "aoT": aoT, "s": s, "done": 0}

        def stage1(sup, t, d, qn, kn, vn):
            """projections -> evac -> scores -> softmax weights -> A@V product."""
            xs, ys, aoT = sup["xs"], sup["ys"], sup["aoT"]
            msl = slice(MT * t, MT * (t + 1))
            ps = {}
            for role, n in (("q", qn), ("k", kn), ("v", vn)):
                ps[n] = pp.tile([128, NF], f32, tag=f"p_{role}", name=f"p_{role}")
            if d == 1:
                srcs = {qn: xs, kn: ys, vn: ys}
            else:
                srcs = {qn: ys, kn: xs, vn: xs}
            sb = {}
            # per projection: 4 K-chunks + bias row, then evacuate right away
            for n in (qn, kn, vn):
                for c in range(4):
                    nc.tensor.matmul(ps[n][:], srcs[n][c][:, msl],
                                     W[n][0][c][:],
                                     start=(c == 0), stop=False)
                nc.tensor.matmul(ps[n][:], ones[:], W[n][1][:],
                                 start=False, stop=True)
                t_sb = qkv_pool.tile([128, NF], bf16, tag=f"s_{n}", name=f"s_{n}")
                if n[0] == "q":
                    nc.scalar.mul(t_sb[:], ps[n][:], 1.0 / SCALE)
                elif n[0] == "k":
                    nc.scalar.copy(t_sb[:], ps[n][:])
                else:
                    # v evacuated d-major: vT[p, d*8+e] so the A@V product's
                    # operands are all innermost-stride-1 (DVE 2x mode)
                    nc.scalar.copy(
                        t_sb[:].rearrange("p (d e) -> p e d", e=H),
                        ps[n][:].rearrange("p (e d) -> p e d", e=H))
                sb[n] = t_sb
            q, k, vT = sb[qn], sb[kn], sb[vn]

            # scores: per-sample q.k dot products, bf16, tree-reduced over d
            prod = big_pool.tile([128, H, H, DH], bf16, tag="bigbuf", name="qkprod")
            q_b = (q[:].rearrange("p (h d) -> p h d", h=H)
                   .unsqueeze(2).broadcast_to([128, H, H, DH]))
            k_b = (k[:].rearrange("p (e d) -> p e d", e=H)
                   .unsqueeze(1).broadcast_to([128, H, H, DH]))
            if sc >= 8:
                nc.vector.tensor_mul(prod[:], q_b, k_b)
            else:
                nc.vector.tensor_mul(prod[:, 0:sc], q_b[:, 0:sc], k_b[:, 0:sc])
                nc.gpsimd.tensor_mul(prod[:, sc:8], q_b[:, sc:8], k_b[:, sc:8])
            tr1 = prod_pool.tile([128, H, H, 32], bf16, tag="trA", name="tr1")
            if tr1_gp:
                nc.vector.tensor_add(tr1[:, 0:tr1_gp], prod[:, 0:tr1_gp, :, 0:32],
                                     prod[:, 0:tr1_gp, :, 32:64])
                nc.gpsimd.tensor_add(tr1[:, tr1_gp:8], prod[:, tr1_gp:8, :, 0:32],
                                     prod[:, tr1_gp:8, :, 32:64])
            else:
                nc.vector.tensor_add(tr1[:], prod[:, :, :, 0:32],
                                     prod[:, :, :, 32:64])
            tr2 = prod_pool.tile([128, H, H, 16], bf16, tag="trB", name="tr2")
            nc.vector.tensor_add(tr2[:], tr1[:, :, :, 0:16], tr1[:, :, :, 16:32])
            tr3 = prod_pool.tile([128, H, H, 8], bf16, tag="trC", name="tr3")
            nc.vector.tensor_add(tr3[:], tr2[:, :, :, 0:8], tr2[:, :, :, 8:16])
            tr4 = prod_pool.tile([128, H, H, 4], bf16, tag="trD", name="tr4")
            nc.vector.tensor_add(tr4[:], tr3[:, :, :, 0:4], tr3[:, :, :, 4:8])
            tr5 = prod_pool.tile([128, H, H, 2], bf16, tag="trE", name="tr5")
            nc.vector.tensor_add(tr5[:], tr4[:, :, :, 0:2], tr4[:, :, :, 2:4])
            s_t = small_pool.tile([128, H * H], bf16, tag="s_t", name="s_t")
            nc.vector.tensor_add(
                s_t[:].rearrange("p (h e) -> p h e", h=H).unsqueeze(3),
                tr5[:, :, :, 0:1], tr5[:, :, :, 1:2])

            # softmax weights (unnormalized exp; normalization deferred to
            # stage2a so the GPSIMD A@V launch sits on a short chain)
            e_t = small_pool.tile([128, H * H], bf16, tag="e_t", name="e_t")
            nc.scalar.activation(e_t[:], s_t[:], EXP)
            z_t = small_pool.tile([128, H], f32, tag="z_t", name="z_t")
            nc.vector.tensor_reduce(
                z_t[:], e_t[:].rearrange("p (h e) -> p h e", h=H),
                axis=X, op=ADD)
            r_t = small_pool.tile([128, H], f32, tag="r_t", name="r_t")
            nc.vector.reciprocal(r_t[:], z_t[:])
            r_bf = small_pool.tile([128, H], bf16, tag="r_bf", name="r_bf")
            nc.scalar.copy(r_bf[:], r_t[:])

            # A@V product, d-major output (d, h, e), all operands stride-1
            p2 = p2_pool.tile([128, DH, H, H], bf16, tag="p2buf", name="avprod")
            a_b = (e_t[:].rearrange("p (h e) -> p h e", h=H)
                   .unsqueeze(1).broadcast_to([128, DH, H, H]))
            v_b = (vT[:].rearrange("p (d e) -> p d e", e=H)
                   .unsqueeze(2).broadcast_to([128, DH, H, H]))
            if hv >= 8:
                nc.vector.tensor_mul(p2[:], a_b, v_b)
            else:
                nc.vector.tensor_mul(p2[:, :, 0:hv, :], a_b[:, :, 0:hv, :],
                                     v_b[:, :, 0:hv, :])
                nc.gpsimd.tensor_mul(p2[:, :, hv:8, :], a_b[:, :, hv:8, :],
                                     v_b[:, :, hv:8, :])
            return {"p2": p2, "r_bf": r_bf, "d": d, "msl": msl,
                    "aoT": aoT[d], "sup": sup}

        def stage2a(st):
            """A@V e-reduction (post GPSIMD): bf16 tree, d-major output."""
            p2 = st["p2"]
            av1 = prod_pool.tile([128, DH, H, 4], bf16, tag="avA", name="av1")
            if av1_gp:
                nc.vector.tensor_add(av1[:, :, 0:av1_gp], p2[:, :, 0:av1_gp, 0:4],
                                     p2[:, :, 0:av1_gp, 4:8])
                nc.gpsimd.tensor_add(av1[:, :, av1_gp:8], p2[:, :, av1_gp:8, 0:4],
                                     p2[:, :, av1_gp:8, 4:8])
            else:
                nc.vector.tensor_add(av1[:], p2[:, :, :, 0:4], p2[:, :, :, 4:8])
            av2 = prod_pool.tile([128, DH, H, 2], bf16, tag="avB", name="av2")
            nc.vector.tensor_add(av2[:], av1[:, :, :, 0:2], av1[:, :, :, 2:4])
            o_u = out_pool.tile([128, NF], bf16, tag="o_u", name="o_u")
            nc.vector.tensor_add(
                o_u[:].rearrange("p (d h) -> p d h", h=H).unsqueeze(3),
                av2[:, :, :, 0:1], av2[:, :, :, 1:2])
            # normalize by 1/z (per sample+head), bf16 2x, d-major
            o_t = out_pool.tile([128, NF], bf16, tag="o_t", name="o_t")
            nc.vector.tensor_mul(
                o_t[:].rearrange("p (d h) -> p d h", h=H),
                o_u[:].rearrange("p (d h) -> p d h", h=H),
                st["r_bf"][:].unsqueeze(1).broadcast_to([128, DH, H]))
            st["o_t"] = o_t

        def stage2b(st):
            """transpose -> aoT staging; out-proj once a super-tile completes."""
            o_t, msl = st["o_t"], st["msl"]
            tp = pt.tile([128, NF], bf16, tag="tp", name="tp")
            for c in range(4):
                nc.tensor.transpose(tp[:, 128 * c:128 * (c + 1)],
                                    o_t[:, 128 * c:128 * (c + 1)],
                                    ident[:])
            nc.scalar.copy(st["aoT"][:, :, msl],
                           tp[:].rearrange("p (c m) -> p c m", c=4))
            sup = st["sup"]
            sup["done"] += 1
            if sup["done"] == 2 * N_MT:
                out_projections(sup)

        def out_projections(sup):
            s, aoT = sup["s"], sup["aoT"]
            ssl = slice(ST * s, ST * (s + 1))
            for d, n in ((1, "o1"), (2, "o2")):
                od = dr["o1T"] if d == 1 else dr["o2T"]
                for o in range(4):
                    op_ps = po.tile([128, NF], f32, tag="op", name="op")
                    for c in range(4):
                        nc.tensor.matmul(op_ps[:],
                                         WO[n][c][:, 128 * o:128 * (o + 1)],
                                         aoT[d][:, c, :],
                                         start=(c == 0), stop=(c == 3))
                    ob = out_pool.tile([128, NF], f32, tag="ob", name="ob")
                    nc.scalar.activation(ob[:], op_ps[:], IDENT_FN,
                                         bias=bo_sb[n][:, o:o + 1], scale=1.0)
                    nc.sync.dma_start(od.ap()[128 * o:128 * (o + 1), ssl], ob[:])

        from collections import deque
        pipe = deque()
        for s in range(N_ST):
            sup = load_supertile(s)
            for t in range(N_MT):
                for d, (qn, kn, vn) in ((1, ("q1", "k1", "v1")),
                                        (2, ("q2", "k2", "v2"))):
                    st = stage1(sup, t, d, qn, kn, vn)
                    pipe.append(st)
                    if len(pipe) >= lag + 1:
                        stage2a(pipe[-(lag + 1)])
                    if len(pipe) >= lag + 2:
                        stage2b(pipe.popleft())
        for st in list(pipe)[-lag:]:
            stage2a(st)
        while pipe:
            stage2b(pipe.popleft())


def _get_module():
    if "nc" in _CACHE:
        return _CACHE["nc"]
    nc = bacc.Bacc("TRN2", target_bir_lowering=False, debug=False,
                   enable_asserts=True, num_devices=NCORES)
    dr = {}
    dr["xT"] = nc.dram_tensor("xT", [NF, BC], f32r, kind="ExternalInput")
    dr["yT"] = nc.dram_tensor("yT", [NF, BC], f32r, kind="ExternalInput")
    for n in WNAMES:
        dr[f"w_{n}"] = nc.dram_tensor(f"w_{n}", [NF + 1, NF], f32r,
                                      kind="ExternalInput")
    dr["wo1T"] = nc.dram_tensor("wo1T", [NF, NF], f32r, kind="ExternalInput")
    dr["wo2T"] = nc.dram_tensor("wo2T", [NF, NF], f32r, kind="ExternalInput")
    dr["bo1c"] = nc.dram_tensor("bo1c", [128, 4], f32, kind="ExternalInput")
    dr["bo2c"] = nc.dram_tensor("bo2c", [128, 4], f32, kind="ExternalInput")
    dr["ones_row"] = nc.dram_tensor("ones_row", [1, MT], f32r,
                                    kind="ExternalInput")
    dr["ident"] = nc.dram_tensor("ident", [128, 128], bf16, kind="ExternalInput")
    dr["o1T"] = nc.dram_tensor("o1T", [NF, BC], f32, kind="ExternalOutput")
    dr["o2T"] = nc.dram_tensor("o2T", [NF, BC], f32, kind="ExternalOutput")

    with tile.TileContext(nc) as tc:
        _emit(nc, tc, dr)
    nc.compile()
    _CACHE["nc"] = nc
    return nc


def _prepare_in_maps(inputs):
    import ml_dtypes

    prot = np.asarray(inputs["protein_features"], dtype=np.float32)
    lig = np.asarray(inputs["ligand_features"], dtype=np.float32)

    shared = {}
    for n in WNAMES:
        wt = np.asarray(inputs[f"W{n}"], dtype=np.float32).T
        bt = np.asarray(inputs[f"b{n}"], dtype=np.float32)[None, :]
        shared[f"w_{n}"] = np.ascontiguousarray(np.concatenate([wt, bt], 0))
    # attention output is d-major (feature d*8+h); permute Wo rows to match
    idx = np.arange(NF)
    perm = (idx % H) * DH + (idx // H)   # dest row d*8+h <- src row h*64+d
    shared["wo1T"] = np.ascontiguousarray(
        np.asarray(inputs["Wo1"], dtype=np.float32).T[perm])
    shared["wo2T"] = np.ascontiguousarray(
        np.asarray(inputs["Wo2"], dtype=np.float32).T[perm])
    shared["bo1c"] = np.ascontiguousarray(
        np.asarray(inputs["bo1"], dtype=np.float32).reshape(4, 128).T)
    shared["bo2c"] = np.ascontiguousarray(
        np.asarray(inputs["bo2"], dtype=np.float32).reshape(4, 128).T)
    shared["ones_row"] = np.ones((1, MT), dtype=np.float32)
    shared["ident"] = np.eye(128, dtype=ml_dtypes.bfloat16)

    in_maps = []
    for c in range(NCORES):
        sl = slice(c * BC, (c + 1) * BC)
        m = dict(shared)
        m["xT"] = np.ascontiguousarray(prot[sl].T)
        m["yT"] = np.ascontiguousarray(lig[sl].T)
        in_maps.append(m)
    return in_maps


def _run(inputs, trace=False, tmpdir=None):
    nc = _get_module()
    in_maps = _prepare_in_maps(inputs)
    res = bass_utils.run_bass_kernel_spmd(
        nc, in_maps, core_ids=list(range(NCORES)), trace=trace, tmpdir=tmpdir)

    p_out = np.empty((B, NF), dtype=np.float32)
    l_out = np.empty((B, NF), dtype=np.float32)
    for c in range(NCORES):
        sl = slice(c * BC, (c + 1) * BC)
        p_out[sl] = res.results[c]["o1T"].T
        l_out[sl] = res.results[c]["o2T"].T
    return (p_out, l_out), res


def kernel(**inputs):
    out, _ = _run(inputs, trace=bool(int(os.environ.get("BIDIR_TRACE", "0"))))
    return out



# revision 57
# speedup vs baseline: 1.1031x; 1.1031x over previous
"""BiDirectionalCrossAttention Trainium2 kernel (8-core data parallel).

Math (per sample m, matching the reference):
  q1 = x @ Wq1.T + bq1   (x = protein)     k1,v1 from y (ligand)
  q2 = y @ Wq2.T + bq2                     k2,v2 from x
  S[h,e]   = q[h,:] . k[e,:] / sqrt(64)    (heads mix: 8x8 scores per sample)
  A        = softmax_e(S)
  out[h,:] = sum_e A[h,e] v[e,:]
  protein_out = out1 @ Wo1.T + bo1 ; ligand_out = out2 @ Wo2.T + bo2

Mapping:
  - batch is sharded 8 ways (4096 samples/core); weights replicated.
  - inputs are passed transposed (xT [512, 4096]); the 128-row K-chunks of
    x.T serve directly as matmul stationary operands; biases ride as a 513th
    row combined with a ones-row K=1 matmul.
  - projections + output projections run on the TensorEngine in float32r.
  - per-sample attention runs with samples on partitions, entirely in bf16
    so every DVE tensor_tensor op hits the 2x perf mode:
      * v is evacuated d-major (vT layout [d*8+e]) so the A@V product's
        operands are all innermost-stride-1;
      * exp output (attention weights) is bf16 and pre-normalized by 1/z
        at [128, 64] (cheap) instead of normalizing the [128, 512] output;
      * the QK d-reduction is a full bf16 binary tree (32->1);
      * attention output is produced d-major and the output-projection
        weights are host-permuted to match, so no normalize pass and a
        bf16 TensorE transpose (1 cyc/row).
  - the big products are split DVE/GPSIMD by head slices to balance engines.
  - per-(tile, direction) work is software-pipelined across the batch.
"""

import os

import numpy as np

import concourse.bacc as bacc
import concourse.mybir as mybir
import concourse.tile as tile
from concourse import bass_utils

B, NF = 32768, 512
H, DH = 8, 64
NCORES = 8
BC = B // NCORES          # samples per core
MT = 128                  # attention tile (samples)
ST = 512                  # projection super-tile (samples)
N_ST = BC // ST
N_MT = ST // MT
SCALE = 8.0               # sqrt(DH)

f32 = mybir.dt.float32
f32r = mybir.dt.float32r
bf16 = mybir.dt.bfloat16

WNAMES = ["q1", "k1", "v1", "q2", "k2", "v2"]

_CACHE: dict = {}


def _iget(name, default):
    return int(os.environ.get(name, str(default)))


def _emit(nc, tc, dr):
    from contextlib import ExitStack

    X = mybir.AxisListType.X
    ADD = mybir.AluOpType.add
    MULT = mybir.AluOpType.mult
    EXP = mybir.ActivationFunctionType.Exp
    IDENT_FN = mybir.ActivationFunctionType.Identity

    sc = _iget("BIDIR_SC_SPLIT", 6)    # qk-product head-lanes on DVE
    tq = _iget("BIDIR_TQ_SPLIT", 7)    # qk-tree head-lanes on DVE
    hv = _iget("BIDIR_AV_SPLIT", 6)    # av-product head-lanes on DVE
    tv = _iget("BIDIR_TV_SPLIT", 7)    # av-tree head-lanes on DVE
    lag = _iget("BIDIR_LAG", 1)
    lag2 = _iget("BIDIR_LAG2", 1)
    prenorm = _iget("BIDIR_PRENORM", 1)
    qk_dma = _iget("BIDIR_QK_DMA", 1)  # level-1 qk reduction on the DMA engines

    # DVE: plain tensor_tensor hits the 2x bf16 perf mode; the
    # scalar_tensor_tensor form would drop to 1x (no perf modes) and the HW
    # compiler additionally limits it to 2-3D access patterns, so both
    # engines use plain TensorTensor for the broadcast-heavy attention ops.
    def vmul(out, a, b):
        nc.vector.tensor_mul(out, a, b)

    def vadd(out, a, b):
        nc.vector.tensor_add(out, a, b)

    def gmul(out, a, b):
        nc.gpsimd.tensor_mul(out, a, b)

    def gadd(out, a, b):
        nc.gpsimd.tensor_add(out, a, b)

    with ExitStack() as ctx:
        wpool = ctx.enter_context(tc.tile_pool(name="weights", bufs=1))
        xpool = ctx.enter_context(tc.tile_pool(name="xstage", bufs=_iget("BIDIR_X_BUFS", 2)))
        qkv_pool = ctx.enter_context(tc.tile_pool(name="qkv", bufs=_iget("BIDIR_QKV_BUFS", 3)))
        big_pool = ctx.enter_context(tc.tile_pool(name="bigp", bufs=_iget("BIDIR_BIG_BUFS", 3)))
        p2_pool = ctx.enter_context(tc.tile_pool(name="p2p", bufs=_iget("BIDIR_P2_BUFS", 2)))
        small_pool = ctx.enter_context(tc.tile_pool(name="small", bufs=_iget("BIDIR_SMALL_BUFS", 4)))
        ao_pool = ctx.enter_context(tc.tile_pool(name="aoT", bufs=_iget("BIDIR_AO_BUFS", 1)))
        ot_pool = ctx.enter_context(tc.tile_pool(name="otb", bufs=_iget("BIDIR_OT_BUFS", 2)))
        out_pool = ctx.enter_context(tc.tile_pool(name="outb", bufs=_iget("BIDIR_OUT_BUFS", 3)))
        pp = ctx.enter_context(tc.tile_pool(name="pproj", bufs=_iget("BIDIR_PP_BUFS", 1), space="PSUM"))
        pt = ctx.enter_context(tc.tile_pool(name="ptrans", bufs=1, space="PSUM"))
        po = ctx.enter_context(tc.tile_pool(name="pout", bufs=1, space="PSUM"))

        # ---- static weights ----
        W = {}
        for n in WNAMES:
            chunks = []
            for c in range(4):
                t = wpool.tile([128, NF], bf16, tag=f"w_{n}_{c}", name=f"w_{n}_{c}")
                nc.sync.dma_start(t[:], dr[f"w_{n}"].ap()[128 * c:128 * (c + 1), :])
                chunks.append(t)
            # bias row + ones stationary in bf16: halves the column footprint
            # and the bias contribution is tiny (0.02-scale), so bf16
            # rounding of it is harmless
            bt = wpool.tile([1, NF], bf16, tag=f"w_{n}_b", name=f"w_{n}_b")
            nc.sync.dma_start(bt[:], dr["wbias"].ap()[WNAMES.index(n):
                                                      WNAMES.index(n) + 1, :])
            W[n] = (chunks, bt[:])
        WO = {}
        for n in ("o1", "o2"):
            WO[n] = []
            for c in range(4):
                t = wpool.tile([128, NF], bf16, tag=f"wo_{n}_{c}", name=f"wo_{n}_{c}")
                nc.sync.dma_start(t[:], dr[f"w{n}T"].ap()[128 * c:128 * (c + 1), :])
                WO[n].append(t)
        bo_sb = {}
        for n in ("o1", "o2"):
            t = wpool.tile([128, 4], f32, tag=f"bo_{n}", name=f"bo_{n}")
            nc.sync.dma_start(t[:], dr[f"b{n}c"].ap())
            bo_sb[n] = t
        ones = wpool.tile([1, MT], bf16, tag="ones", name="ones")
        nc.sync.dma_start(ones[:], dr["ones_row"].ap())
        ident = wpool.tile([128, 128], bf16, tag="ident", name="ident")
        nc.sync.dma_start(ident[:], dr["ident"].ap())

        def load_supertile(s):
            ssl = slice(ST * s, ST * (s + 1))
            xs, ys = [], []
            for c in range(4):
                xt = xpool.tile([128, ST], bf16, tag=f"xs{c}", name=f"xs{c}")
                nc.sync.dma_start(xt[:], dr["xT"].ap()[128 * c:128 * (c + 1), ssl])
                xs.append(xt)
                yt = xpool.tile([128, ST], bf16, tag=f"ys{c}", name=f"ys{c}")
                nc.sync.dma_start(yt[:], dr["yT"].ap()[128 * c:128 * (c + 1), ssl])
                ys.append(yt)
            aoT = {1: ao_pool.tile([128, 4, ST], bf16, tag="aoT1", name="aoT1"),
                   2: ao_pool.tile([128, 4, ST], bf16, tag="aoT2", name="aoT2")}
            return {"xs": xs, "ys": ys, "aoT": aoT, "s": s, "done": 0}

        # The two directions are fused into single DVE/Pool ops with the
        # direction index i interleaved in the INNERMOST stride:
        #   q_sb [128, (h d i)]   k_sb [128, (e d i)]   v_sb [128, (d e i)]
        #   prod [128, m, (h e d2 i)]  p2 [128, (d h e i)]  o_t [128, (d h i)]
        # so every operand view collapses to <=3 free dims ((d i)/(e i)
        # merge), which the HW TensorTensor pattern requires, while staying
        # innermost-packed for the DVE 2x mode (incl. the final tree levels).

        def stage1a(sup, t):
            """Both directions' projections + evacs (PE/Act only).
            q,k of both directions are projected first so the DVE qk
            product can start before the v evacs land."""
            xs, ys = sup["xs"], sup["ys"]
            msl = slice(MT * t, MT * (t + 1))
            q_sb = qkv_pool.tile([128, NF, 2], bf16, tag="s_q", name="q_sb")
            k_sb = qkv_pool.tile([128, NF, 2], bf16, tag="s_k", name="k_sb")
            v_sb = qkv_pool.tile([128, NF, 2], bf16, tag="s_v", name="v_sb")
            plan = [("q1", 0, q_sb), ("k1", 0, k_sb),
                    ("q2", 1, q_sb), ("k2", 1, k_sb),
                    ("v1", 0, v_sb), ("v2", 1, v_sb)]
            for n, di, t_sb in plan:
                fr_x = (n[0] == "q") == (di == 0)
                srcn = xs if fr_x else ys
                psn = pp.tile([128, NF], f32, tag=f"p_{n[0]}{di}",
                              name=f"p_{n[0]}{di}")
                for c in range(4):
                    nc.tensor.matmul(psn[:], srcn[c][:, msl],
                                     W[n][0][c][:],
                                     start=(c == 0), stop=False)
                nc.tensor.matmul(psn[:], ones[:], W[n][1],
                                 start=False, stop=True)
                if n[0] == "q":
                    nc.scalar.mul(t_sb[:, :, di], psn[:], 1.0 / SCALE)
                elif n[0] == "k":
                    nc.scalar.copy(t_sb[:, :, di], psn[:])
                else:
                    # v stored d-major (d e i); psum is (e d)
                    nc.scalar.copy(
                        t_sb[:, :, di].rearrange("p (d e) -> p e d", e=H),
                        psn[:].rearrange("p (e d) -> p e d", e=H))
            return {"q_sb": q_sb, "k_sb": k_sb, "v_sb": v_sb,
                    "msl": msl, "sup": sup}

        def stage1b(st):
            """qk products (DVE head-lanes [0:sc), GPSIMD the rest).
            Written in two d-half blocks (m OUTERMOST) so the 64->32
            reduction is one contiguous accumulate-DMA."""
            q_sb, k_sb = st["q_sb"], st["k_sb"]
            W2 = DH  # merged (d_half i) width = 32*2
            prod = big_pool.tile([128, 2, H, H, W2], bf16,
                                 tag="bigbuf", name="qkprod")

            def qb_view(m, h0, h1):
                return (q_sb[:].rearrange("p (h m w) i -> p h (m w) i",
                                          h=H, m=2)
                        .rearrange("p h w i -> p h (w i)")
                        [:, h0:h1, W2 * m: W2 * (m + 1)]
                        .unsqueeze(2).broadcast_to([128, h1 - h0, H, W2]))

            def kb_view(m, h0, h1):
                return (k_sb[:].rearrange("p (e m w) i -> p e (m w) i",
                                          e=H, m=2)
                        .rearrange("p e w i -> p e (w i)")
                        [:, :, W2 * m: W2 * (m + 1)]
                        .unsqueeze(1).broadcast_to([128, h1 - h0, H, W2]))

            for m in (0, 1):
                if sc >= 8:
                    vmul(prod[:, m], qb_view(m, 0, 8), kb_view(m, 0, 8))
                else:
                    vmul(prod[:, m, 0:sc], qb_view(m, 0, sc),
                         kb_view(m, 0, sc))
                    gmul(prod[:, m, sc:8], qb_view(m, sc, 8),
                         kb_view(m, sc, 8))
            st["prod"] = prod

        def stage1b2(st):
            """64->32 qk reduction: contiguous accumulate-DMA (SWDGE), or
            DVE/Pool lanes when BIDIR_QK_DMA=0."""
            prod = st["prod"]
            if qk_dma:
                nc.gpsimd.dma_start(
                    prod[:, 0].rearrange("p h e w -> p (h e w)"),
                    prod[:, 1].rearrange("p h e w -> p (h e w)"),
                    accum_op=ADD)
            else:
                if tq >= 8:
                    vadd(prod[:, 0], prod[:, 0], prod[:, 1])
                else:
                    vadd(prod[:, 0, 0:tq], prod[:, 0, 0:tq],
                         prod[:, 1, 0:tq])
                    gadd(prod[:, 0, tq:8], prod[:, 0, tq:8],
                         prod[:, 1, tq:8])

        def stage1c(st):
            """qk tree -> softmax -> A@V products.  All tree levels are
            contiguous-slice in-place adds on the (d i)-merged axis, so
            even the last level stays innermost-packed (2x)."""
            prod, v_sb = st["prod"], st["v_sb"]
            tr = prod[:, 0]
            w2 = DH // 2
            while w2 >= 2:
                if tq >= 8:
                    vadd(tr[:, :, :, 0:w2], tr[:, :, :, 0:w2],
                         tr[:, :, :, w2:2 * w2])
                else:
                    vadd(tr[:, 0:tq, :, 0:w2], tr[:, 0:tq, :, 0:w2],
                         tr[:, 0:tq, :, w2:2 * w2])
                    gadd(tr[:, tq:8, :, 0:w2], tr[:, tq:8, :, 0:w2],
                         tr[:, tq:8, :, w2:2 * w2])
                w2 //= 2

            # softmax weights: A = exp(S) * (1/z)
            e_t = small_pool.tile([128, H, H, 2], bf16, tag="e_t", name="e_t")
            nc.scalar.activation(e_t[:], tr[:, :, :, 0:2], EXP)
            z_t = small_pool.tile([128, 2 * H], f32, tag="z_t", name="z_t")
            nc.vector.tensor_reduce(
                z_t[:].rearrange("p (h i) -> p h i", i=2),
                e_t[:].rearrange("p h e i -> p h i e"), axis=X, op=ADD)
            r_bf = small_pool.tile([128, 2 * H], bf16, tag="r_bf", name="r_bf")
            with nc.allow_low_precision(reason="1/z consumed in bf16 anyway"):
                nc.vector.reciprocal(r_bf[:], z_t[:])
            if prenorm:
                a_t = small_pool.tile([128, H, H, 2], bf16, tag="a_t",
                                      name="a_t")
                vmul(a_t[:], e_t[:],
                     r_bf[:].rearrange("p (h i) -> p h i", i=2)
                     .unsqueeze(2).broadcast_to([128, H, H, 2]))
            else:
                a_t = e_t

            # A@V product: p2 [128, (d h e i)], operands innermost-packed
            p2 = p2_pool.tile([128, DH, H, 2 * H], bf16, tag="p2buf",
                              name="avprod")
            a_b = (a_t[:].rearrange("p h e i -> p h (e i)")
                   .unsqueeze(1).broadcast_to([128, DH, H, 2 * H]))
            v_b = (v_sb[:].rearrange("p (d e) i -> p d (e i)", e=H)
                   .unsqueeze(2).broadcast_to([128, DH, H, 2 * H]))
            if hv >= 8:
                vmul(p2[:], a_b, v_b)
            elif hv <= 0:
                gmul(p2[:], a_b, v_b)
            else:
                vmul(p2[:, :, 0:hv], a_b[:, :, 0:hv], v_b[:, :, 0:hv])
                gmul(p2[:, :, hv:8], a_b[:, :, hv:8], v_b[:, :, hv:8])
            st["p2"] = p2
            st["r_bf"] = r_bf

        def stage2(st):
            """A@V e-reduction (contiguous in-place tree on the (e i) axis)
            -> o_t [128, (d h i)] (DVE/Pool)."""
            p2, msl, sup = st["p2"], st["msl"], st["sup"]
            for w2 in (8, 4):
                if tv >= 8:
                    vadd(p2[:, :, :, 0:w2], p2[:, :, :, 0:w2],
                         p2[:, :, :, w2:2 * w2])
                else:
                    vadd(p2[:, :, 0:tv, 0:w2], p2[:, :, 0:tv, 0:w2],
                         p2[:, :, 0:tv, w2:2 * w2])
                    gadd(p2[:, :, tv:8, 0:w2], p2[:, :, tv:8, 0:w2],
                         p2[:, :, tv:8, w2:2 * w2])
            o_t = ot_pool.tile([128, NF * 2], bf16, tag="o_t", name="o_t")
            o_v = o_t[:].rearrange("p (d h i) -> p d h i", h=H, i=2)
            if tv >= 8:
                vadd(o_v, p2[:, :, :, 0:2], p2[:, :, :, 2:4])
            else:
                vadd(o_v[:, :, 0:tv], p2[:, :, 0:tv, 0:2],
                     p2[:, :, 0:tv, 2:4])
                gadd(o_v[:, :, tv:8], p2[:, :, tv:8, 0:2],
                     p2[:, :, tv:8, 2:4])
            if not prenorm:
                o_n = ot_pool.tile([128, NF * 2], bf16, tag="o_n", name="o_n")
                vmul(o_n[:].rearrange("p (d h i) -> p d h i", h=H, i=2), o_v,
                     st["r_bf"][:].rearrange("p (h i) -> p h i", i=2)
                     .unsqueeze(1).broadcast_to([128, DH, H, 2]))
                o_t = o_n
            st["o_t"] = o_t

        def stage3(st):
            """transposes -> aoT staging (PE/Act only, lagged further so the
            slow DVE/Pool chain can't head-of-line-block projections);
            out-proj once a super-tile completes."""
            o_t, msl, sup = st["o_t"], st["msl"], st["sup"]
            o_4 = o_t[:].rearrange("p (d h i) -> p d h i", h=H, i=2)
            for di in range(2):
                tp = pt.tile([128, NF], bf16, tag="tp", name="tp")
                for c in range(4):
                    nc.tensor.transpose(
                        tp[:, 128 * c:128 * (c + 1)]
                        .rearrange("p (d h) -> p d h", h=H),
                        o_4[:, 16 * c:16 * (c + 1), :, di],
                        ident[:])
                nc.scalar.copy(sup["aoT"][di + 1][:, :, msl],
                               tp[:].rearrange("p (c m) -> p c m", c=4))
            sup["done"] += 1
            if sup["done"] == N_MT:
                out_projections(sup)

        def out_projections(sup):
            s, aoT = sup["s"], sup["aoT"]
            ssl = slice(ST * s, ST * (s + 1))
            for d, n in ((1, "o1"), (2, "o2")):
                od = dr["o1T"] if d == 1 else dr["o2T"]
                for o in range(4):
                    op_ps = po.tile([128, NF], f32, tag="op", name="op")
                    for c in range(4):
                        nc.tensor.matmul(op_ps[:],
                                         WO[n][c][:, 128 * o:128 * (o + 1)],
                                         aoT[d][:, c, :],
                                         start=(c == 0), stop=(c == 3))
                    ob = out_pool.tile([128, NF], f32, tag="ob", name="ob")
                    nc.scalar.activation(ob[:], op_ps[:], IDENT_FN,
                                         bias=bo_sb[n][:, o:o + 1], scale=1.0)
                    # store via the Act HWDGE queue: keeps the SP queue free
                    # for input loads, and the producer is Act anyway
                    nc.scalar.dma_start(od.ap()[128 * o:128 * (o + 1), ssl],
                                        ob[:])

        from collections import deque
        pipe0, pipe1, pipe2 = deque(), deque(), deque()

        def step(st=None):
            # 5-slot software pipeline, one tile per slot:
            #   A(t)   proj+evac            (PE/Act)
            #   B(t)   qk products          (DVE/Pool)
            #   C(t-1) l1-accum-DMA + qk tree + softmax + av products
            #   D(t-2) av trees -> o_t      (DVE/Pool)
            #   E(t-3) transposes + staging (+ out-proj)
            # The accum-DMA at the head of C sees products issued a full
            # iteration earlier, so its SEQ wait on the Pool queue is ~nil,
            # and every engine always has a tile's worth of ready work.
            if st is not None:
                stage1b(st)
                pipe0.append(st)
            if len(pipe0) > lag or (st is None and pipe0):
                stC = pipe0.popleft()
                stage1b2(stC)
                stage1c(stC)
                pipe1.append(stC)
            if len(pipe1) > lag or (st is None and pipe1):
                stD = pipe1.popleft()
                stage2(stD)
                pipe2.append(stD)
            if len(pipe2) > lag2 or (st is None and pipe2):
                stage3(pipe2.popleft())

        for s in range(N_ST):
            sup = load_supertile(s)
            for t in range(N_MT):
                step(stage1a(sup, t))
        while pipe0 or pipe1 or pipe2:
            step()


def _get_module():
    if "nc" in _CACHE:
        return _CACHE["nc"]
    nc = bacc.Bacc("TRN2", target_bir_lowering=False, debug=False,
                   enable_asserts=True, num_devices=NCORES)
    dr = {}
    dr["xT"] = nc.dram_tensor("xT", [NF, BC], bf16, kind="ExternalInput")
    dr["yT"] = nc.dram_tensor("yT", [NF, BC], bf16, kind="ExternalInput")
    for n in WNAMES:
        dr[f"w_{n}"] = nc.dram_tensor(f"w_{n}", [NF, NF], bf16,
                                      kind="ExternalInput")
    dr["wbias"] = nc.dram_tensor("wbias", [len(WNAMES), NF], bf16,
                                 kind="ExternalInput")
    dr["wo1T"] = nc.dram_tensor("wo1T", [NF, NF], bf16, kind="ExternalInput")
    dr["wo2T"] = nc.dram_tensor("wo2T", [NF, NF], bf16, kind="ExternalInput")
    dr["bo1c"] = nc.dram_tensor("bo1c", [128, 4], f32, kind="ExternalInput")
    dr["bo2c"] = nc.dram_tensor("bo2c", [128, 4], f32, kind="ExternalInput")
    dr["ones_row"] = nc.dram_tensor("ones_row", [1, MT], bf16,
                                    kind="ExternalInput")
    dr["ident"] = nc.dram_tensor("ident", [128, 128], bf16, kind="ExternalInput")
    dr["o1T"] = nc.dram_tensor("o1T", [NF, BC], f32, kind="ExternalOutput")
    dr["o2T"] = nc.dram_tensor("o2T", [NF, BC], f32, kind="ExternalOutput")

    with tile.TileContext(nc) as tc:
        _emit(nc, tc, dr)
    nc.compile()
    _CACHE["nc"] = nc
    return nc


def _prepare_in_maps(inputs):
    import ml_dtypes

    prot = np.asarray(inputs["protein_features"], dtype=np.float32)
    lig = np.asarray(inputs["ligand_features"], dtype=np.float32)

    shared = {}
    for n in WNAMES:
        wt = np.asarray(inputs[f"W{n}"], dtype=np.float32).T
        shared[f"w_{n}"] = np.ascontiguousarray(wt).astype(ml_dtypes.bfloat16)
    shared["wbias"] = np.stack(
        [np.asarray(inputs[f"b{n}"], dtype=np.float32) for n in WNAMES]
    ).astype(ml_dtypes.bfloat16)
    # attention output is d-major (feature d*8+h); permute Wo rows to match
    idx = np.arange(NF)
    perm = (idx % H) * DH + (idx // H)   # dest row d*8+h <- src row h*64+d
    shared["wo1T"] = np.ascontiguousarray(
        np.asarray(inputs["Wo1"], dtype=np.float32).T[perm]).astype(
        ml_dtypes.bfloat16)
    shared["wo2T"] = np.ascontiguousarray(
        np.asarray(inputs["Wo2"], dtype=np.float32).T[perm]).astype(
        ml_dtypes.bfloat16)
    shared["bo1c"] = np.ascontiguousarray(
        np.asarray(inputs["bo1"], dtype=np.float32).reshape(4, 128).T)
    shared["bo2c"] = np.ascontiguousarray(
        np.asarray(inputs["bo2"], dtype=np.float32).reshape(4, 128).T)
    shared["ones_row"] = np.ones((1, MT), dtype=ml_dtypes.bfloat16)
    shared["ident"] = np.eye(128, dtype=ml_dtypes.bfloat16)

    in_maps = []
    for c in range(NCORES):
        sl = slice(c * BC, (c + 1) * BC)
        m = dict(shared)
        m["xT"] = np.ascontiguousarray(prot[sl].T).astype(ml_dtypes.bfloat16)
        m["yT"] = np.ascontiguousarray(lig[sl].T).astype(ml_dtypes.bfloat16)
        in_maps.append(m)
    return in_maps


def _run(inputs, trace=False, tmpdir=None):
    nc = _get_module()
    in_maps = _prepare_in_maps(inputs)
    res = bass_utils.run_bass_kernel_spmd(
        nc, in_maps, core_ids=list(range(NCORES)), trace=trace, tmpdir=tmpdir)

    p_out = np.empty((B, NF), dtype=np.float32)
    l_out = np.empty((B, NF), dtype=np.float32)
    for c in range(NCORES):
        sl = slice(c * BC, (c + 1) * BC)
        p_out[sl] = res.results[c]["o1T"].T
        l_out[sl] = res.results[c]["o2T"].T
    return (p_out, l_out), res


def kernel(**inputs):
    out, _ = _run(inputs, trace=bool(int(os.environ.get("BIDIR_TRACE", "0"))))
    return out



# revision 59
# speedup vs baseline: 1.1433x; 1.0364x over previous
"""BiDirectionalCrossAttention Trainium2 kernel (8-core data parallel).

Math (per sample m, matching the reference):
  q1 = x @ Wq1.T + bq1   (x = protein)     k1,v1 from y (ligand)
  q2 = y @ Wq2.T + bq2                     k2,v2 from x
  S[h,e]   = q[h,:] . k[e,:] / sqrt(64)    (heads mix: 8x8 scores per sample)
  A        = softmax_e(S)
  out[h,:] = sum_e A[h,e] v[e,:]
  protein_out = out1 @ Wo1.T + bo1 ; ligand_out = out2 @ Wo2.T + bo2

Mapping:
  - batch is sharded 8 ways (4096 samples/core); weights replicated.
  - inputs are passed transposed (xT [512, 4096]); the 128-row K-chunks of
    x.T serve directly as matmul stationary operands; biases ride as a 513th
    row combined with a ones-row K=1 matmul.
  - projections + output projections run on the TensorEngine in float32r.
  - per-sample attention runs with samples on partitions, entirely in bf16
    so every DVE tensor_tensor op hits the 2x perf mode:
      * v is evacuated d-major (vT layout [d*8+e]) so the A@V product's
        operands are all innermost-stride-1;
      * exp output (attention weights) is bf16 and pre-normalized by 1/z
        at [128, 64] (cheap) instead of normalizing the [128, 512] output;
      * the QK d-reduction is a full bf16 binary tree (32->1);
      * attention output is produced d-major and the output-projection
        weights are host-permuted to match, so no normalize pass and a
        bf16 TensorE transpose (1 cyc/row).
  - the big products are split DVE/GPSIMD by head slices to balance engines.
  - per-(tile, direction) work is software-pipelined across the batch.
"""

import os

import numpy as np

import concourse.bacc as bacc
import concourse.mybir as mybir
import concourse.tile as tile
from concourse import bass_utils

B, NF = 32768, 512
H, DH = 8, 64
NCORES = 8
BC = B // NCORES          # samples per core
MT = 128                  # attention tile (samples)
ST = 512                  # projection super-tile (samples)
N_ST = BC // ST
N_MT = ST // MT
SCALE = 8.0               # sqrt(DH)

f32 = mybir.dt.float32
f32r = mybir.dt.float32r
bf16 = mybir.dt.bfloat16

WNAMES = ["q1", "k1", "v1", "q2", "k2", "v2"]

_CACHE: dict = {}


def _iget(name, default):
    return int(os.environ.get(name, str(default)))


def _emit(nc, tc, dr):
    from contextlib import ExitStack

    X = mybir.AxisListType.X
    ADD = mybir.AluOpType.add
    MULT = mybir.AluOpType.mult
    EXP = mybir.ActivationFunctionType.Exp
    IDENT_FN = mybir.ActivationFunctionType.Identity

    sc = _iget("BIDIR_SC_SPLIT", 6)    # qk-product head-lanes on DVE
    tq = _iget("BIDIR_TQ_SPLIT", 8)    # qk-tree head-lanes on DVE
    hv = _iget("BIDIR_AV_SPLIT", 6)    # av-product head-lanes on DVE
    tv = _iget("BIDIR_TV_SPLIT", 8)    # av-tree head-lanes on DVE
    lag = _iget("BIDIR_LAG", 1)
    lag2 = _iget("BIDIR_LAG2", 1)
    prenorm = _iget("BIDIR_PRENORM", 1)
    # the SWDGE accumulate-DMA path faults real HW (cost model liked it);
    # default OFF — the 64->32 level runs on the engine lanes instead
    qk_dma = _iget("BIDIR_QK_DMA", 0)

    # DVE: plain tensor_tensor hits the 2x bf16 perf mode; the
    # scalar_tensor_tensor form would drop to 1x (no perf modes) and the HW
    # compiler additionally limits it to 2-3D access patterns, so both
    # engines use plain TensorTensor for the broadcast-heavy attention ops.
    def vmul(out, a, b):
        nc.vector.tensor_mul(out, a, b)

    def vadd(out, a, b):
        nc.vector.tensor_add(out, a, b)

    def gmul(out, a, b):
        nc.gpsimd.tensor_mul(out, a, b)

    def gadd(out, a, b):
        nc.gpsimd.tensor_add(out, a, b)

    with ExitStack() as ctx:
        wpool = ctx.enter_context(tc.tile_pool(name="weights", bufs=1))
        xpool = ctx.enter_context(tc.tile_pool(name="xstage", bufs=_iget("BIDIR_X_BUFS", 2)))
        qkv_pool = ctx.enter_context(tc.tile_pool(name="qkv", bufs=_iget("BIDIR_QKV_BUFS", 3)))
        big_pool = ctx.enter_context(tc.tile_pool(name="bigp", bufs=_iget("BIDIR_BIG_BUFS", 3)))
        p2_pool = ctx.enter_context(tc.tile_pool(name="p2p", bufs=_iget("BIDIR_P2_BUFS", 2)))
        small_pool = ctx.enter_context(tc.tile_pool(name="small", bufs=_iget("BIDIR_SMALL_BUFS", 4)))
        ao_pool = ctx.enter_context(tc.tile_pool(name="aoT", bufs=_iget("BIDIR_AO_BUFS", 1)))
        ot_pool = ctx.enter_context(tc.tile_pool(name="otb", bufs=_iget("BIDIR_OT_BUFS", 2)))
        out_pool = ctx.enter_context(tc.tile_pool(name="outb", bufs=_iget("BIDIR_OUT_BUFS", 3)))
        pp = ctx.enter_context(tc.tile_pool(name="pproj", bufs=_iget("BIDIR_PP_BUFS", 1), space="PSUM"))
        pt = ctx.enter_context(tc.tile_pool(name="ptrans", bufs=1, space="PSUM"))
        po = ctx.enter_context(tc.tile_pool(name="pout", bufs=1, space="PSUM"))

        # ---- static weights ----
        W = {}
        for n in WNAMES:
            chunks = []
            for c in range(4):
                t = wpool.tile([128, NF], bf16, tag=f"w_{n}_{c}", name=f"w_{n}_{c}")
                nc.sync.dma_start(t[:], dr[f"w_{n}"].ap()[128 * c:128 * (c + 1), :])
                chunks.append(t)
            # bias row + ones stationary in bf16: halves the column footprint
            # and the bias contribution is tiny (0.02-scale), so bf16
            # rounding of it is harmless
            bt = wpool.tile([1, NF], bf16, tag=f"w_{n}_b", name=f"w_{n}_b")
            nc.sync.dma_start(bt[:], dr["wbias"].ap()[WNAMES.index(n):
                                                      WNAMES.index(n) + 1, :])
            W[n] = (chunks, bt[:])
        WO = {}
        for n in ("o1", "o2"):
            WO[n] = []
            for c in range(4):
                t = wpool.tile([128, NF], bf16, tag=f"wo_{n}_{c}", name=f"wo_{n}_{c}")
                nc.sync.dma_start(t[:], dr[f"w{n}T"].ap()[128 * c:128 * (c + 1), :])
                WO[n].append(t)
        bo_sb = {}
        for n in ("o1", "o2"):
            t = wpool.tile([128, 4], f32, tag=f"bo_{n}", name=f"bo_{n}")
            nc.sync.dma_start(t[:], dr[f"b{n}c"].ap())
            bo_sb[n] = t
        ones = wpool.tile([1, MT], bf16, tag="ones", name="ones")
        nc.sync.dma_start(ones[:], dr["ones_row"].ap())
        ident = wpool.tile([128, 128], bf16, tag="ident", name="ident")
        nc.sync.dma_start(ident[:], dr["ident"].ap())

        def load_supertile(s):
            ssl = slice(ST * s, ST * (s + 1))
            xs, ys = [], []
            for c in range(4):
                xt = xpool.tile([128, ST], bf16, tag=f"xs{c}", name=f"xs{c}")
                nc.sync.dma_start(xt[:], dr["xT"].ap()[128 * c:128 * (c + 1), ssl])
                xs.append(xt)
                yt = xpool.tile([128, ST], bf16, tag=f"ys{c}", name=f"ys{c}")
                nc.sync.dma_start(yt[:], dr["yT"].ap()[128 * c:128 * (c + 1), ssl])
                ys.append(yt)
            aoT = {1: ao_pool.tile([128, 4, ST], bf16, tag="aoT1", name="aoT1"),
                   2: ao_pool.tile([128, 4, ST], bf16, tag="aoT2", name="aoT2")}
            return {"xs": xs, "ys": ys, "aoT": aoT, "s": s, "done": 0}

        # The two directions are fused into single DVE/Pool ops with the
        # direction index i interleaved in the INNERMOST stride:
        #   q_sb [128, (h d i)]   k_sb [128, (e d i)]   v_sb [128, (d e i)]
        #   prod [128, m, (h e d2 i)]  p2 [128, (d h e i)]  o_t [128, (d h i)]
        # so every operand view collapses to <=3 free dims ((d i)/(e i)
        # merge), which the HW TensorTensor pattern requires, while staying
        # innermost-packed for the DVE 2x mode (incl. the final tree levels).

        def stage1a(sup, t):
            """Both directions' projections + evacs (PE/Act only).
            q,k of both directions are projected first so the DVE qk
            product can start before the v evacs land."""
            xs, ys = sup["xs"], sup["ys"]
            msl = slice(MT * t, MT * (t + 1))
            q_sb = qkv_pool.tile([128, NF, 2], bf16, tag="s_q", name="q_sb")
            k_sb = qkv_pool.tile([128, NF, 2], bf16, tag="s_k", name="k_sb")
            v_sb = qkv_pool.tile([128, NF, 2], bf16, tag="s_v", name="v_sb")
            plan = [("q1", 0, q_sb), ("k1", 0, k_sb),
                    ("q2", 1, q_sb), ("k2", 1, k_sb),
                    ("v1", 0, v_sb), ("v2", 1, v_sb)]
            for n, di, t_sb in plan:
                fr_x = (n[0] == "q") == (di == 0)
                srcn = xs if fr_x else ys
                psn = pp.tile([128, NF], f32, tag=f"p_{n[0]}{di}",
                              name=f"p_{n[0]}{di}")
                for c in range(4):
                    nc.tensor.matmul(psn[:], srcn[c][:, msl],
                                     W[n][0][c][:],
                                     start=(c == 0), stop=False)
                nc.tensor.matmul(psn[:], ones[:], W[n][1],
                                 start=False, stop=True)
                if n[0] == "q":
                    nc.scalar.mul(t_sb[:, :, di], psn[:], 1.0 / SCALE)
                elif n[0] == "k":
                    nc.scalar.copy(t_sb[:, :, di], psn[:])
                else:
                    # v stored d-major (d e i); psum is (e d)
                    nc.scalar.copy(
                        t_sb[:, :, di].rearrange("p (d e) -> p e d", e=H),
                        psn[:].rearrange("p (e d) -> p e d", e=H))
            return {"q_sb": q_sb, "k_sb": k_sb, "v_sb": v_sb,
                    "msl": msl, "sup": sup}

        def stage1b(st):
            """qk products (DVE head-lanes [0:sc), GPSIMD the rest).
            Written in two d-half blocks (m OUTERMOST) so the 64->32
            reduction is one contiguous accumulate-DMA."""
            q_sb, k_sb = st["q_sb"], st["k_sb"]
            W2 = DH  # merged (d_half i) width = 32*2
            prod = big_pool.tile([128, 2, H, H, W2], bf16,
                                 tag="bigbuf", name="qkprod")

            def qb_view(m, h0, h1):
                return (q_sb[:].rearrange("p (h m w) i -> p h (m w) i",
                                          h=H, m=2)
                        .rearrange("p h w i -> p h (w i)")
                        [:, h0:h1, W2 * m: W2 * (m + 1)]
                        .unsqueeze(2).broadcast_to([128, h1 - h0, H, W2]))

            def kb_view(m, h0, h1):
                return (k_sb[:].rearrange("p (e m w) i -> p e (m w) i",
                                          e=H, m=2)
                        .rearrange("p e w i -> p e (w i)")
                        [:, :, W2 * m: W2 * (m + 1)]
                        .unsqueeze(1).broadcast_to([128, h1 - h0, H, W2]))

            for m in (0, 1):
                if sc >= 8:
                    vmul(prod[:, m], qb_view(m, 0, 8), kb_view(m, 0, 8))
                else:
                    vmul(prod[:, m, 0:sc], qb_view(m, 0, sc),
                         kb_view(m, 0, sc))
                    gmul(prod[:, m, sc:8], qb_view(m, sc, 8),
                         kb_view(m, sc, 8))
            st["prod"] = prod

        def stage1b2(st):
            """64->32 qk reduction: contiguous accumulate-DMA (SWDGE), or
            DVE/Pool lanes when BIDIR_QK_DMA=0."""
            prod = st["prod"]
            if qk_dma:
                nc.gpsimd.dma_start(
                    prod[:, 0].rearrange("p h e w -> p (h e w)"),
                    prod[:, 1].rearrange("p h e w -> p (h e w)"),
                    accum_op=ADD)
            else:
                if tq >= 8:
                    vadd(prod[:, 0], prod[:, 0], prod[:, 1])
                else:
                    vadd(prod[:, 0, 0:tq], prod[:, 0, 0:tq],
                         prod[:, 1, 0:tq])
                    gadd(prod[:, 0, tq:8], prod[:, 0, tq:8],
                         prod[:, 1, tq:8])

        def stage1c(st):
            """qk tree -> softmax -> A@V products.  All tree levels are
            contiguous-slice in-place adds on the (d i)-merged axis, so
            even the last level stays innermost-packed (2x)."""
            prod, v_sb = st["prod"], st["v_sb"]
            tr = prod[:, 0]
            w2 = DH // 2
            while w2 >= 2:
                if tq >= 8:
                    vadd(tr[:, :, :, 0:w2], tr[:, :, :, 0:w2],
                         tr[:, :, :, w2:2 * w2])
                else:
                    vadd(tr[:, 0:tq, :, 0:w2], tr[:, 0:tq, :, 0:w2],
                         tr[:, 0:tq, :, w2:2 * w2])
                    gadd(tr[:, tq:8, :, 0:w2], tr[:, tq:8, :, 0:w2],
                         tr[:, tq:8, :, w2:2 * w2])
                w2 //= 2

            # softmax weights: A = exp(S) * (1/z)
            e_t = small_pool.tile([128, H, H, 2], bf16, tag="e_t", name="e_t")
            nc.scalar.activation(e_t[:], tr[:, :, :, 0:2], EXP)
            z_t = small_pool.tile([128, 2 * H], f32, tag="z_t", name="z_t")
            nc.vector.tensor_reduce(
                z_t[:].rearrange("p (h i) -> p h i", i=2),
                e_t[:].rearrange("p h e i -> p h i e"), axis=X, op=ADD)
            r_bf = small_pool.tile([128, 2 * H], bf16, tag="r_bf", name="r_bf")
            with nc.allow_low_precision(reason="1/z consumed in bf16 anyway"):
                nc.vector.reciprocal(r_bf[:], z_t[:])
            if prenorm:
                a_t = small_pool.tile([128, H, H, 2], bf16, tag="a_t",
                                      name="a_t")
                vmul(a_t[:], e_t[:],
                     r_bf[:].rearrange("p (h i) -> p h i", i=2)
                     .unsqueeze(2).broadcast_to([128, H, H, 2]))
            else:
                a_t = e_t

            # A@V product: p2 [128, (d h e i)], operands innermost-packed
            p2 = p2_pool.tile([128, DH, H, 2 * H], bf16, tag="p2buf",
                              name="avprod")
            a_b = (a_t[:].rearrange("p h e i -> p h (e i)")
                   .unsqueeze(1).broadcast_to([128, DH, H, 2 * H]))
            v_b = (v_sb[:].rearrange("p (d e) i -> p d (e i)", e=H)
                   .unsqueeze(2).broadcast_to([128, DH, H, 2 * H]))
            if hv >= 8:
                vmul(p2[:], a_b, v_b)
            elif hv <= 0:
                gmul(p2[:], a_b, v_b)
            else:
                vmul(p2[:, :, 0:hv], a_b[:, :, 0:hv], v_b[:, :, 0:hv])
                gmul(p2[:, :, hv:8], a_b[:, :, hv:8], v_b[:, :, hv:8])
            st["p2"] = p2
            st["r_bf"] = r_bf

        def stage2(st):
            """A@V e-reduction (contiguous in-place tree on the (e i) axis)
            -> o_t [128, (d h i)] (DVE/Pool)."""
            p2, msl, sup = st["p2"], st["msl"], st["sup"]
            for w2 in (8, 4):
                if tv >= 8:
                    vadd(p2[:, :, :, 0:w2], p2[:, :, :, 0:w2],
                         p2[:, :, :, w2:2 * w2])
                else:
                    vadd(p2[:, :, 0:tv, 0:w2], p2[:, :, 0:tv, 0:w2],
                         p2[:, :, 0:tv, w2:2 * w2])
                    gadd(p2[:, :, tv:8, 0:w2], p2[:, :, tv:8, 0:w2],
                         p2[:, :, tv:8, w2:2 * w2])
            o_t = ot_pool.tile([128, NF * 2], bf16, tag="o_t", name="o_t")
            o_v = o_t[:].rearrange("p (d h i) -> p d h i", h=H, i=2)
            if tv >= 8:
                vadd(o_v, p2[:, :, :, 0:2], p2[:, :, :, 2:4])
            else:
                vadd(o_v[:, :, 0:tv], p2[:, :, 0:tv, 0:2],
                     p2[:, :, 0:tv, 2:4])
                gadd(o_v[:, :, tv:8], p2[:, :, tv:8, 0:2],
                     p2[:, :, tv:8, 2:4])
            if not prenorm:
                o_n = ot_pool.tile([128, NF * 2], bf16, tag="o_n", name="o_n")
                vmul(o_n[:].rearrange("p (d h i) -> p d h i", h=H, i=2), o_v,
                     st["r_bf"][:].rearrange("p (h i) -> p h i", i=2)
                     .unsqueeze(1).broadcast_to([128, DH, H, 2]))
                o_t = o_n
            st["o_t"] = o_t

        def stage3(st):
            """transposes -> aoT staging (PE/Act only, lagged further so the
            slow DVE/Pool chain can't head-of-line-block projections);
            out-proj once a super-tile completes."""
            o_t, msl, sup = st["o_t"], st["msl"], st["sup"]
            o_4 = o_t[:].rearrange("p (d h i) -> p d h i", h=H, i=2)
            for di in range(2):
                tp = pt.tile([128, NF], bf16, tag="tp", name="tp")
                for c in range(4):
                    nc.tensor.transpose(
                        tp[:, 128 * c:128 * (c + 1)]
                        .rearrange("p (d h) -> p d h", h=H),
                        o_4[:, 16 * c:16 * (c + 1), :, di],
                        ident[:])
                nc.scalar.copy(sup["aoT"][di + 1][:, :, msl],
                               tp[:].rearrange("p (c m) -> p c m", c=4))
            sup["done"] += 1
            if sup["done"] == N_MT:
                out_projections(sup)

        def out_projections(sup):
            s, aoT = sup["s"], sup["aoT"]
            ssl = slice(ST * s, ST * (s + 1))
            for d, n in ((1, "o1"), (2, "o2")):
                od = dr["o1T"] if d == 1 else dr["o2T"]
                for o in range(4):
                    op_ps = po.tile([128, NF], f32, tag="op", name="op")
                    for c in range(4):
                        nc.tensor.matmul(op_ps[:],
                                         WO[n][c][:, 128 * o:128 * (o + 1)],
                                         aoT[d][:, c, :],
                                         start=(c == 0), stop=(c == 3))
                    ob = out_pool.tile([128, NF], f32, tag="ob", name="ob")
                    nc.scalar.activation(ob[:], op_ps[:], IDENT_FN,
                                         bias=bo_sb[n][:, o:o + 1], scale=1.0)
                    # store via the Act HWDGE queue: keeps the SP queue free
                    # for input loads, and the producer is Act anyway
                    nc.scalar.dma_start(od.ap()[128 * o:128 * (o + 1), ssl],
                                        ob[:])

        from collections import deque
        pipe0, pipe1, pipe2 = deque(), deque(), deque()

        def step(st=None):
            # 5-slot software pipeline, one tile per slot:
            #   A(t)   proj+evac            (PE/Act)
            #   B(t)   qk products          (DVE/Pool)
            #   C(t-1) l1-accum-DMA + qk tree + softmax + av products
            #   D(t-2) av trees -> o_t      (DVE/Pool)
            #   E(t-3) transposes + staging (+ out-proj)
            # The accum-DMA at the head of C sees products issued a full
            # iteration earlier, so its SEQ wait on the Pool queue is ~nil,
            # and every engine always has a tile's worth of ready work.
            if st is not None:
                stage1b(st)
                pipe0.append(st)
            if len(pipe0) > lag or (st is None and pipe0):
                stC = pipe0.popleft()
                stage1b2(stC)
                stage1c(stC)
                pipe1.append(stC)
            if len(pipe1) > lag or (st is None and pipe1):
                stD = pipe1.popleft()
                stage2(stD)
                pipe2.append(stD)
            if len(pipe2) > lag2 or (st is None and pipe2):
                stage3(pipe2.popleft())

        for s in range(N_ST):
            sup = load_supertile(s)
            for t in range(N_MT):
                step(stage1a(sup, t))
        while pipe0 or pipe1 or pipe2:
            step()


def _get_module():
    if "nc" in _CACHE:
        return _CACHE["nc"]
    nc = bacc.Bacc("TRN2", target_bir_lowering=False, debug=False,
                   enable_asserts=True, num_devices=NCORES)
    dr = {}
    dr["xT"] = nc.dram_tensor("xT", [NF, BC], bf16, kind="ExternalInput")
    dr["yT"] = nc.dram_tensor("yT", [NF, BC], bf16, kind="ExternalInput")
    for n in WNAMES:
        dr[f"w_{n}"] = nc.dram_tensor(f"w_{n}", [NF, NF], bf16,
                                      kind="ExternalInput")
    dr["wbias"] = nc.dram_tensor("wbias", [len(WNAMES), NF], bf16,
                                 kind="ExternalInput")
    dr["wo1T"] = nc.dram_tensor("wo1T", [NF, NF], bf16, kind="ExternalInput")
    dr["wo2T"] = nc.dram_tensor("wo2T", [NF, NF], bf16, kind="ExternalInput")
    dr["bo1c"] = nc.dram_tensor("bo1c", [128, 4], f32, kind="ExternalInput")
    dr["bo2c"] = nc.dram_tensor("bo2c", [128, 4], f32, kind="ExternalInput")
    dr["ones_row"] = nc.dram_tensor("ones_row", [1, MT], bf16,
                                    kind="ExternalInput")
    dr["ident"] = nc.dram_tensor("ident", [128, 128], bf16, kind="ExternalInput")
    dr["o1T"] = nc.dram_tensor("o1T", [NF, BC], f32, kind="ExternalOutput")
    dr["o2T"] = nc.dram_tensor("o2T", [NF, BC], f32, kind="ExternalOutput")

    with tile.TileContext(nc) as tc:
        _emit(nc, tc, dr)
    nc.compile()
    _CACHE["nc"] = nc
    return nc


def _prepare_in_maps(inputs):
    import ml_dtypes

    prot = np.asarray(inputs["protein_features"], dtype=np.float32)
    lig = np.asarray(inputs["ligand_features"], dtype=np.float32)

    shared = {}
    for n in WNAMES:
        wt = np.asarray(inputs[f"W{n}"], dtype=np.float32).T
        shared[f"w_{n}"] = np.ascontiguousarray(wt).astype(ml_dtypes.bfloat16)
    shared["wbias"] = np.stack(
        [np.asarray(inputs[f"b{n}"], dtype=np.float32) for n in WNAMES]
    ).astype(ml_dtypes.bfloat16)
    # attention output is d-major (feature d*8+h); permute Wo rows to match
    idx = np.arange(NF)
    perm = (idx % H) * DH + (idx // H)   # dest row d*8+h <- src row h*64+d
    shared["wo1T"] = np.ascontiguousarray(
        np.asarray(inputs["Wo1"], dtype=np.float32).T[perm]).astype(
        ml_dtypes.bfloat16)
    shared["wo2T"] = np.ascontiguousarray(
        np.asarray(inputs["Wo2"], dtype=np.float32).T[perm]).astype(
        ml_dtypes.bfloat16)
    shared["bo1c"] = np.ascontiguousarray(
        np.asarray(inputs["bo1"], dtype=np.float32).reshape(4, 128).T)
    shared["bo2c"] = np.ascontiguousarray(
        np.asarray(inputs["bo2"], dtype=np.float32).reshape(4, 128).T)
    shared["ones_row"] = np.ones((1, MT), dtype=ml_dtypes.bfloat16)
    shared["ident"] = np.eye(128, dtype=ml_dtypes.bfloat16)

    in_maps = []
    for c in range(NCORES):
        sl = slice(c * BC, (c + 1) * BC)
        m = dict(shared)
        m["xT"] = np.ascontiguousarray(prot[sl].T).astype(ml_dtypes.bfloat16)
        m["yT"] = np.ascontiguousarray(lig[sl].T).astype(ml_dtypes.bfloat16)
        in_maps.append(m)
    return in_maps


def _run(inputs, trace=False, tmpdir=None):
    nc = _get_module()
    in_maps = _prepare_in_maps(inputs)
    res = bass_utils.run_bass_kernel_spmd(
        nc, in_maps, core_ids=list(range(NCORES)), trace=trace, tmpdir=tmpdir)

    p_out = np.empty((B, NF), dtype=np.float32)
    l_out = np.empty((B, NF), dtype=np.float32)
    for c in range(NCORES):
        sl = slice(c * BC, (c + 1) * BC)
        p_out[sl] = res.results[c]["o1T"].T
        l_out[sl] = res.results[c]["o2T"].T
    return (p_out, l_out), res


def kernel(**inputs):
    out, _ = _run(inputs, trace=bool(int(os.environ.get("BIDIR_TRACE", "0"))))
    return out



# revision 62
# speedup vs baseline: 1.1891x; 1.0401x over previous
"""BiDirectionalCrossAttention Trainium2 kernel (8-core data parallel).

Math (per sample m, matching the reference):
  q1 = x @ Wq1.T + bq1   (x = protein)     k1,v1 from y (ligand)
  q2 = y @ Wq2.T + bq2                     k2,v2 from x
  S[h,e]   = q[h,:] . k[e,:] / sqrt(64)    (heads mix: 8x8 scores per sample)
  A        = softmax_e(S)
  out[h,:] = sum_e A[h,e] v[e,:]
  protein_out = out1 @ Wo1.T + bo1 ; ligand_out = out2 @ Wo2.T + bo2

Mapping:
  - batch is sharded 8 ways (4096 samples/core); weights replicated.
  - inputs are passed transposed and pre-cast to bf16 (xT [512, 4096]); the
    128-row K-chunks of x.T serve directly as matmul stationary operands;
    biases ride on a bf16 ones-row K=1 matmul (all matmul inputs bf16 —
    the HW compiler rejects mixed 32/16-bit Matmult inputs).
  - per-sample attention runs with samples on partitions, entirely in bf16.
    BOTH directions are fused into each DVE/GPSIMD op with the direction
    index interleaved in the innermost stride:
      q [128,(h d i)]  k [128,(e d i)]  v [128,(d e i)]  out [128,(d h i)]
    so every operand view collapses to <=3 free dims (the HW TensorTensor
    limit) via (d i)/(e i) merges while staying innermost-packed for the
    DVE 2x perf mode — including the final tree levels, which fold the two
    directions' adjacent elements.
  - softmax weights are pre-normalized by 1/z at [128,128] instead of
    normalizing the [128,1024] output; reductions are in-place contiguous-
    slice binary trees inside the product buffers (no extra tiles).
  - work is split DVE/GPSIMD by head lanes (whole product+tree chains per
    lane) and software-pipelined over 5 stages per 128-sample tile:
    A proj+evac (PE/Act), B qk products, C qk tree+softmax+av products,
    D av tree, E transpose+staging+out-proj — each a tile behind the last,
    so cross-engine waits are covered by older tiles' ready work.
"""

import os

import numpy as np

import concourse.bacc as bacc
import concourse.mybir as mybir
import concourse.tile as tile
from concourse import bass_utils

B, NF = 32768, 512
H, DH = 8, 64
NCORES = 8
BC = B // NCORES          # samples per core
MT = 128                  # attention tile (samples)
ST = 512                  # projection super-tile (samples)
N_ST = BC // ST
N_MT = ST // MT
SCALE = 8.0               # sqrt(DH)

f32 = mybir.dt.float32
f32r = mybir.dt.float32r
bf16 = mybir.dt.bfloat16

WNAMES = ["q1", "k1", "v1", "q2", "k2", "v2"]

_CACHE: dict = {}


def _iget(name, default):
    return int(os.environ.get(name, str(default)))


def _emit(nc, tc, dr):
    from contextlib import ExitStack

    X = mybir.AxisListType.X
    ADD = mybir.AluOpType.add
    MULT = mybir.AluOpType.mult
    EXP = mybir.ActivationFunctionType.Exp
    IDENT_FN = mybir.ActivationFunctionType.Identity

    sc = _iget("BIDIR_SC_SPLIT", 6)    # qk-product head-lanes on DVE
    tq = _iget("BIDIR_TQ_SPLIT", 8)    # qk-tree head-lanes on DVE
    hv = _iget("BIDIR_AV_SPLIT", 6)    # av-product head-lanes on DVE
    tv = _iget("BIDIR_TV_SPLIT", 5)    # av-tree head-lanes on DVE
    lag = _iget("BIDIR_LAG", 1)
    lag2 = _iget("BIDIR_LAG2", 1)
    prenorm = _iget("BIDIR_PRENORM", 1)
    # the SWDGE accumulate-DMA path faults real HW (cost model liked it);
    # default OFF — the 64->32 level runs on the engine lanes instead
    qk_dma = _iget("BIDIR_QK_DMA", 0)

    # DVE: plain tensor_tensor hits the 2x bf16 perf mode; the
    # scalar_tensor_tensor form would drop to 1x (no perf modes) and the HW
    # compiler additionally limits it to 2-3D access patterns, so both
    # engines use plain TensorTensor for the broadcast-heavy attention ops.
    def vmul(out, a, b):
        nc.vector.tensor_mul(out, a, b)

    def vadd(out, a, b):
        nc.vector.tensor_add(out, a, b)

    def gmul(out, a, b):
        nc.gpsimd.tensor_mul(out, a, b)

    def gadd(out, a, b):
        nc.gpsimd.tensor_add(out, a, b)

    with ExitStack() as ctx:
        wpool = ctx.enter_context(tc.tile_pool(name="weights", bufs=1))
        xpool = ctx.enter_context(tc.tile_pool(name="xstage", bufs=_iget("BIDIR_X_BUFS", 2)))
        qkv_pool = ctx.enter_context(tc.tile_pool(name="qkv", bufs=_iget("BIDIR_QKV_BUFS", 3)))
        big_pool = ctx.enter_context(tc.tile_pool(name="bigp", bufs=_iget("BIDIR_BIG_BUFS", 3)))
        p2_pool = ctx.enter_context(tc.tile_pool(name="p2p", bufs=_iget("BIDIR_P2_BUFS", 2)))
        small_pool = ctx.enter_context(tc.tile_pool(name="small", bufs=_iget("BIDIR_SMALL_BUFS", 4)))
        ao_pool = ctx.enter_context(tc.tile_pool(name="aoT", bufs=_iget("BIDIR_AO_BUFS", 1)))
        ot_pool = ctx.enter_context(tc.tile_pool(name="otb", bufs=_iget("BIDIR_OT_BUFS", 2)))
        out_pool = ctx.enter_context(tc.tile_pool(name="outb", bufs=_iget("BIDIR_OUT_BUFS", 3)))
        pp = ctx.enter_context(tc.tile_pool(name="pproj", bufs=_iget("BIDIR_PP_BUFS", 1), space="PSUM"))
        pt = ctx.enter_context(tc.tile_pool(name="ptrans", bufs=1, space="PSUM"))
        po = ctx.enter_context(tc.tile_pool(name="pout", bufs=1, space="PSUM"))

        # ---- static weights ----
        W = {}
        for n in WNAMES:
            chunks = []
            for c in range(4):
                t = wpool.tile([128, NF], bf16, tag=f"w_{n}_{c}", name=f"w_{n}_{c}")
                nc.sync.dma_start(t[:], dr[f"w_{n}"].ap()[128 * c:128 * (c + 1), :])
                chunks.append(t)
            # bias row + ones stationary in bf16: halves the column footprint
            # and the bias contribution is tiny (0.02-scale), so bf16
            # rounding of it is harmless
            bt = wpool.tile([1, NF], bf16, tag=f"w_{n}_b", name=f"w_{n}_b")
            nc.sync.dma_start(bt[:], dr["wbias"].ap()[WNAMES.index(n):
                                                      WNAMES.index(n) + 1, :])
            W[n] = (chunks, bt[:])
        WO = {}
        for n in ("o1", "o2"):
            WO[n] = []
            for c in range(4):
                t = wpool.tile([128, NF], bf16, tag=f"wo_{n}_{c}", name=f"wo_{n}_{c}")
                nc.sync.dma_start(t[:], dr[f"w{n}T"].ap()[128 * c:128 * (c + 1), :])
                WO[n].append(t)
        bo_sb = {}
        for n in ("o1", "o2"):
            t = wpool.tile([128, 4], f32, tag=f"bo_{n}", name=f"bo_{n}")
            nc.sync.dma_start(t[:], dr[f"b{n}c"].ap())
            bo_sb[n] = t
        ones = wpool.tile([1, MT], bf16, tag="ones", name="ones")
        nc.sync.dma_start(ones[:], dr["ones_row"].ap())
        ident = wpool.tile([128, 128], bf16, tag="ident", name="ident")
        nc.sync.dma_start(ident[:], dr["ident"].ap())

        def load_supertile(s):
            ssl = slice(ST * s, ST * (s + 1))
            xs, ys = [], []
            for c in range(4):
                xt = xpool.tile([128, ST], bf16, tag=f"xs{c}", name=f"xs{c}")
                nc.sync.dma_start(xt[:], dr["xT"].ap()[128 * c:128 * (c + 1), ssl])
                xs.append(xt)
                yt = xpool.tile([128, ST], bf16, tag=f"ys{c}", name=f"ys{c}")
                nc.sync.dma_start(yt[:], dr["yT"].ap()[128 * c:128 * (c + 1), ssl])
                ys.append(yt)
            aoT = {1: ao_pool.tile([128, 4, ST], bf16, tag="aoT1", name="aoT1"),
                   2: ao_pool.tile([128, 4, ST], bf16, tag="aoT2", name="aoT2")}
            return {"xs": xs, "ys": ys, "aoT": aoT, "s": s, "done": 0}

        # The two directions are fused into single DVE/Pool ops with the
        # direction index i interleaved in the INNERMOST stride:
        #   q_sb [128, (h d i)]   k_sb [128, (e d i)]   v_sb [128, (d e i)]
        #   prod [128, m, (h e d2 i)]  p2 [128, (d h e i)]  o_t [128, (d h i)]
        # so every operand view collapses to <=3 free dims ((d i)/(e i)
        # merge), which the HW TensorTensor pattern requires, while staying
        # innermost-packed for the DVE 2x mode (incl. the final tree levels).

        def stage1a(sup, t):
            """Both directions' projections + evacs (PE/Act only).
            q,k of both directions are projected first so the DVE qk
            product can start before the v evacs land."""
            xs, ys = sup["xs"], sup["ys"]
            msl = slice(MT * t, MT * (t + 1))
            q_sb = qkv_pool.tile([128, NF, 2], bf16, tag="s_q", name="q_sb")
            k_sb = qkv_pool.tile([128, NF, 2], bf16, tag="s_k", name="k_sb")
            v_sb = qkv_pool.tile([128, NF, 2], bf16, tag="s_v", name="v_sb")
            plan = [("q1", 0, q_sb), ("k1", 0, k_sb),
                    ("q2", 1, q_sb), ("k2", 1, k_sb),
                    ("v1", 0, v_sb), ("v2", 1, v_sb)]
            for n, di, t_sb in plan:
                fr_x = (n[0] == "q") == (di == 0)
                srcn = xs if fr_x else ys
                psn = pp.tile([128, NF], f32, tag=f"p_{n[0]}{di}",
                              name=f"p_{n[0]}{di}")
                for c in range(4):
                    nc.tensor.matmul(psn[:], srcn[c][:, msl],
                                     W[n][0][c][:],
                                     start=(c == 0), stop=False)
                nc.tensor.matmul(psn[:], ones[:], W[n][1],
                                 start=False, stop=True)
                if n[0] == "q":
                    nc.scalar.mul(t_sb[:, :, di], psn[:], 1.0 / SCALE)
                elif n[0] == "k":
                    nc.scalar.copy(t_sb[:, :, di], psn[:])
                else:
                    # v stored d-major (d e i); psum is (e d)
                    nc.scalar.copy(
                        t_sb[:, :, di].rearrange("p (d e) -> p e d", e=H),
                        psn[:].rearrange("p (e d) -> p e d", e=H))
            return {"q_sb": q_sb, "k_sb": k_sb, "v_sb": v_sb,
                    "msl": msl, "sup": sup}

        def stage1b(st):
            """qk products (DVE head-lanes [0:sc), GPSIMD the rest).
            Written in two d-half blocks (m OUTERMOST) so the 64->32
            reduction is one contiguous accumulate-DMA."""
            q_sb, k_sb = st["q_sb"], st["k_sb"]
            W2 = DH  # merged (d_half i) width = 32*2
            prod = big_pool.tile([128, 2, H, H, W2], bf16,
                                 tag="bigbuf", name="qkprod")

            def qb_view(m, h0, h1):
                return (q_sb[:].rearrange("p (h m w) i -> p h (m w) i",
                                          h=H, m=2)
                        .rearrange("p h w i -> p h (w i)")
                        [:, h0:h1, W2 * m: W2 * (m + 1)]
                        .unsqueeze(2).broadcast_to([128, h1 - h0, H, W2]))

            def kb_view(m, h0, h1):
                return (k_sb[:].rearrange("p (e m w) i -> p e (m w) i",
                                          e=H, m=2)
                        .rearrange("p e w i -> p e (w i)")
                        [:, :, W2 * m: W2 * (m + 1)]
                        .unsqueeze(1).broadcast_to([128, h1 - h0, H, W2]))

            for m in (0, 1):
                if sc >= 8:
                    vmul(prod[:, m], qb_view(m, 0, 8), kb_view(m, 0, 8))
                else:
                    vmul(prod[:, m, 0:sc], qb_view(m, 0, sc),
                         kb_view(m, 0, sc))
                    gmul(prod[:, m, sc:8], qb_view(m, sc, 8),
                         kb_view(m, sc, 8))
            st["prod"] = prod

        def stage1b2(st):
            """64->32 qk reduction: contiguous accumulate-DMA (SWDGE), or
            DVE/Pool lanes when BIDIR_QK_DMA=0."""
            prod = st["prod"]
            if qk_dma:
                nc.gpsimd.dma_start(
                    prod[:, 0].rearrange("p h e w -> p (h e w)"),
                    prod[:, 1].rearrange("p h e w -> p (h e w)"),
                    accum_op=ADD)
            else:
                if tq >= 8:
                    vadd(prod[:, 0], prod[:, 0], prod[:, 1])
                else:
                    vadd(prod[:, 0, 0:tq], prod[:, 0, 0:tq],
                         prod[:, 1, 0:tq])
                    gadd(prod[:, 0, tq:8], prod[:, 0, tq:8],
                         prod[:, 1, tq:8])

        def stage1c(st):
            """qk tree -> softmax -> A@V products.  All tree levels are
            contiguous-slice in-place adds on the (d i)-merged axis, so
            even the last level stays innermost-packed (2x)."""
            prod, v_sb = st["prod"], st["v_sb"]
            tr = prod[:, 0]
            w2 = DH // 2
            while w2 >= 2:
                if tq >= 8:
                    vadd(tr[:, :, :, 0:w2], tr[:, :, :, 0:w2],
                         tr[:, :, :, w2:2 * w2])
                else:
                    vadd(tr[:, 0:tq, :, 0:w2], tr[:, 0:tq, :, 0:w2],
                         tr[:, 0:tq, :, w2:2 * w2])
                    gadd(tr[:, tq:8, :, 0:w2], tr[:, tq:8, :, 0:w2],
                         tr[:, tq:8, :, w2:2 * w2])
                w2 //= 2

            # softmax weights: A = exp(S) * (1/z)
            e_t = small_pool.tile([128, H, H, 2], bf16, tag="e_t", name="e_t")
            nc.scalar.activation(e_t[:], tr[:, :, :, 0:2], EXP)
            z_t = small_pool.tile([128, 2 * H], f32, tag="z_t", name="z_t")
            nc.vector.tensor_reduce(
                z_t[:].rearrange("p (h i) -> p h i", i=2),
                e_t[:].rearrange("p h e i -> p h i e"), axis=X, op=ADD)
            r_bf = small_pool.tile([128, 2 * H], bf16, tag="r_bf", name="r_bf")
            with nc.allow_low_precision(reason="1/z consumed in bf16 anyway"):
                nc.vector.reciprocal(r_bf[:], z_t[:])
            if prenorm:
                a_t = small_pool.tile([128, H, H, 2], bf16, tag="a_t",
                                      name="a_t")
                vmul(a_t[:], e_t[:],
                     r_bf[:].rearrange("p (h i) -> p h i", i=2)
                     .unsqueeze(2).broadcast_to([128, H, H, 2]))
            else:
                a_t = e_t

            # A@V product: p2 [128, (d h e i)], operands innermost-packed
            p2 = p2_pool.tile([128, DH, H, 2 * H], bf16, tag="p2buf",
                              name="avprod")
            a_b = (a_t[:].rearrange("p h e i -> p h (e i)")
                   .unsqueeze(1).broadcast_to([128, DH, H, 2 * H]))
            v_b = (v_sb[:].rearrange("p (d e) i -> p d (e i)", e=H)
                   .unsqueeze(2).broadcast_to([128, DH, H, 2 * H]))
            if hv >= 8:
                vmul(p2[:], a_b, v_b)
            elif hv <= 0:
                gmul(p2[:], a_b, v_b)
            else:
                vmul(p2[:, :, 0:hv], a_b[:, :, 0:hv], v_b[:, :, 0:hv])
                gmul(p2[:, :, hv:8], a_b[:, :, hv:8], v_b[:, :, hv:8])
            st["p2"] = p2
            st["r_bf"] = r_bf

        def stage2(st):
            """A@V e-reduction (contiguous in-place tree on the (e i) axis)
            -> o_t [128, (d h i)] (DVE/Pool)."""
            p2, msl, sup = st["p2"], st["msl"], st["sup"]
            for w2 in (8, 4):
                if tv >= 8:
                    vadd(p2[:, :, :, 0:w2], p2[:, :, :, 0:w2],
                         p2[:, :, :, w2:2 * w2])
                else:
                    vadd(p2[:, :, 0:tv, 0:w2], p2[:, :, 0:tv, 0:w2],
                         p2[:, :, 0:tv, w2:2 * w2])
                    gadd(p2[:, :, tv:8, 0:w2], p2[:, :, tv:8, 0:w2],
                         p2[:, :, tv:8, w2:2 * w2])
            o_t = ot_pool.tile([128, NF * 2], bf16, tag="o_t", name="o_t")
            o_v = o_t[:].rearrange("p (d h i) -> p d h i", h=H, i=2)
            if tv >= 8:
                vadd(o_v, p2[:, :, :, 0:2], p2[:, :, :, 2:4])
            else:
                vadd(o_v[:, :, 0:tv], p2[:, :, 0:tv, 0:2],
                     p2[:, :, 0:tv, 2:4])
                gadd(o_v[:, :, tv:8], p2[:, :, tv:8, 0:2],
                     p2[:, :, tv:8, 2:4])
            if not prenorm:
                o_n = ot_pool.tile([128, NF * 2], bf16, tag="o_n", name="o_n")
                vmul(o_n[:].rearrange("p (d h i) -> p d h i", h=H, i=2), o_v,
                     st["r_bf"][:].rearrange("p (h i) -> p h i", i=2)
                     .unsqueeze(1).broadcast_to([128, DH, H, 2]))
                o_t = o_n
            st["o_t"] = o_t

        def stage3(st):
            """transposes -> aoT staging (PE/Act only, lagged further so the
            slow DVE/Pool chain can't head-of-line-block projections);
            out-proj once a super-tile completes."""
            o_t, msl, sup = st["o_t"], st["msl"], st["sup"]
            o_4 = o_t[:].rearrange("p (d h i) -> p d h i", h=H, i=2)
            for di in range(2):
                tp = pt.tile([128, NF], bf16, tag="tp", name="tp")
                for c in range(4):
                    nc.tensor.transpose(
                        tp[:, 128 * c:128 * (c + 1)]
                        .rearrange("p (d h) -> p d h", h=H),
                        o_4[:, 16 * c:16 * (c + 1), :, di],
                        ident[:])
                nc.scalar.copy(sup["aoT"][di + 1][:, :, msl],
                               tp[:].rearrange("p (c m) -> p c m", c=4))
            sup["done"] += 1
            if sup["done"] == N_MT:
                out_projections(sup)

        def out_projections(sup):
            s, aoT = sup["s"], sup["aoT"]
            ssl = slice(ST * s, ST * (s + 1))
            for d, n in ((1, "o1"), (2, "o2")):
                od = dr["o1T"] if d == 1 else dr["o2T"]
                for o in range(4):
                    op_ps = po.tile([128, NF], f32, tag="op", name="op")
                    for c in range(4):
                        nc.tensor.matmul(op_ps[:],
                                         WO[n][c][:, 128 * o:128 * (o + 1)],
                                         aoT[d][:, c, :],
                                         start=(c == 0), stop=(c == 3))
                    ob = out_pool.tile([128, NF], f32, tag="ob", name="ob")
                    nc.scalar.activation(ob[:], op_ps[:], IDENT_FN,
                                         bias=bo_sb[n][:, o:o + 1], scale=1.0)
                    # store via the Act HWDGE queue: keeps the SP queue free
                    # for input loads, and the producer is Act anyway
                    nc.scalar.dma_start(od.ap()[128 * o:128 * (o + 1), ssl],
                                        ob[:])

        from collections import deque
        pipe0, pipe1, pipe2 = deque(), deque(), deque()

        order = _iget("BIDIR_ORDER", 1)

        def step(st=None):
            # 5-slot software pipeline, one tile per slot:
            #   A(t)   proj+evac            (PE/Act)
            #   B(t)   qk products          (DVE/Pool)
            #   C(t-1) qk l1+tree + softmax + av products
            #   D(t-2) av trees -> o_t      (DVE/Pool)
            #   E(t-3) transposes + staging (+ out-proj)
            # With BIDIR_ORDER=1 the older tiles' C/D blocks are emitted
            # BEFORE this tile's B products, so the DVE/Pool queues hold
            # ready work (t-1, t-2) in front of the products that must wait
            # for this tile's Act evacs.
            if st is not None and not order:
                stage1b(st)
                pipe0.append(st)
            if len(pipe0) > lag or (st is None and pipe0):
                stC = pipe0.popleft()
                stage1b2(stC)
                stage1c(stC)
                pipe1.append(stC)
            if len(pipe1) > lag or (st is None and pipe1):
                stD = pipe1.popleft()
                stage2(stD)
                pipe2.append(stD)
            if st is not None and order:
                stage1b(st)
                pipe0.append(st)
            if len(pipe2) > lag2 or (st is None and pipe2):
                stage3(pipe2.popleft())

        for s in range(N_ST):
            sup = load_supertile(s)
            for t in range(N_MT):
                step(stage1a(sup, t))
        while pipe0 or pipe1 or pipe2:
            step()


def _get_module():
    if "nc" in _CACHE:
        return _CACHE["nc"]
    nc = bacc.Bacc("TRN2", target_bir_lowering=False, debug=False,
                   enable_asserts=True, num_devices=NCORES)
    dr = {}
    dr["xT"] = nc.dram_tensor("xT", [NF, BC], bf16, kind="ExternalInput")
    dr["yT"] = nc.dram_tensor("yT", [NF, BC], bf16, kind="ExternalInput")
    for n in WNAMES:
        dr[f"w_{n}"] = nc.dram_tensor(f"w_{n}", [NF, NF], bf16,
                                      kind="ExternalInput")
    dr["wbias"] = nc.dram_tensor("wbias", [len(WNAMES), NF], bf16,
                                 kind="ExternalInput")
    dr["wo1T"] = nc.dram_tensor("wo1T", [NF, NF], bf16, kind="ExternalInput")
    dr["wo2T"] = nc.dram_tensor("wo2T", [NF, NF], bf16, kind="ExternalInput")
    dr["bo1c"] = nc.dram_tensor("bo1c", [128, 4], f32, kind="ExternalInput")
    dr["bo2c"] = nc.dram_tensor("bo2c", [128, 4], f32, kind="ExternalInput")
    dr["ones_row"] = nc.dram_tensor("ones_row", [1, MT], bf16,
                                    kind="ExternalInput")
    dr["ident"] = nc.dram_tensor("ident", [128, 128], bf16, kind="ExternalInput")
    dr["o1T"] = nc.dram_tensor("o1T", [NF, BC], f32, kind="ExternalOutput")
    dr["o2T"] = nc.dram_tensor("o2T", [NF, BC], f32, kind="ExternalOutput")

    with tile.TileContext(nc) as tc:
        _emit(nc, tc, dr)
    nc.compile()
    _CACHE["nc"] = nc
    return nc


def _prepare_in_maps(inputs):
    import ml_dtypes

    prot = np.asarray(inputs["protein_features"], dtype=np.float32)
    lig = np.asarray(inputs["ligand_features"], dtype=np.float32)

    shared = {}
    for n in WNAMES:
        wt = np.asarray(inputs[f"W{n}"], dtype=np.float32).T
        shared[f"w_{n}"] = np.ascontiguousarray(wt).astype(ml_dtypes.bfloat16)
    shared["wbias"] = np.stack(
        [np.asarray(inputs[f"b{n}"], dtype=np.float32) for n in WNAMES]
    ).astype(ml_dtypes.bfloat16)
    # attention output is d-major (feature d*8+h); permute Wo rows to match
    idx = np.arange(NF)
    perm = (idx % H) * DH + (idx // H)   # dest row d*8+h <- src row h*64+d
    shared["wo1T"] = np.ascontiguousarray(
        np.asarray(inputs["Wo1"], dtype=np.float32).T[perm]).astype(
        ml_dtypes.bfloat16)
    shared["wo2T"] = np.ascontiguousarray(
        np.asarray(inputs["Wo2"], dtype=np.float32).T[perm]).astype(
        ml_dtypes.bfloat16)
    shared["bo1c"] = np.ascontiguousarray(
        np.asarray(inputs["bo1"], dtype=np.float32).reshape(4, 128).T)
    shared["bo2c"] = np.ascontiguousarray(
        np.asarray(inputs["bo2"], dtype=np.float32).reshape(4, 128).T)
    shared["ones_row"] = np.ones((1, MT), dtype=ml_dtypes.bfloat16)
    shared["ident"] = np.eye(128, dtype=ml_dtypes.bfloat16)

    in_maps = []
    for c in range(NCORES):
        sl = slice(c * BC, (c + 1) * BC)
        m = dict(shared)
        m["xT"] = np.ascontiguousarray(prot[sl].T).astype(ml_dtypes.bfloat16)
        m["yT"] = np.ascontiguousarray(lig[sl].T).astype(ml_dtypes.bfloat16)
        in_maps.append(m)
    return in_maps


def _run(inputs, trace=False, tmpdir=None):
    nc = _get_module()
    in_maps = _prepare_in_maps(inputs)
    res = bass_utils.run_bass_kernel_spmd(
        nc, in_maps, core_ids=list(range(NCORES)), trace=trace, tmpdir=tmpdir)

    p_out = np.empty((B, NF), dtype=np.float32)
    l_out = np.empty((B, NF), dtype=np.float32)
    for c in range(NCORES):
        sl = slice(c * BC, (c + 1) * BC)
        p_out[sl] = res.results[c]["o1T"].T
        l_out[sl] = res.results[c]["o2T"].T
    return (p_out, l_out), res


def kernel(**inputs):
    out, _ = _run(inputs, trace=bool(int(os.environ.get("BIDIR_TRACE", "0"))))
    return out



# revision 63
# speedup vs baseline: 1.2118x; 1.0191x over previous
"""BiDirectionalCrossAttention Trainium2 kernel (8-core data parallel).

Math (per sample m, matching the reference):
  q1 = x @ Wq1.T + bq1   (x = protein)     k1,v1 from y (ligand)
  q2 = y @ Wq2.T + bq2                     k2,v2 from x
  S[h,e]   = q[h,:] . k[e,:] / sqrt(64)    (heads mix: 8x8 scores per sample)
  A        = softmax_e(S)
  out[h,:] = sum_e A[h,e] v[e,:]
  protein_out = out1 @ Wo1.T + bo1 ; ligand_out = out2 @ Wo2.T + bo2

Mapping:
  - batch is sharded 8 ways (4096 samples/core); weights replicated.
  - inputs are passed transposed and pre-cast to bf16 (xT [512, 4096]); the
    128-row K-chunks of x.T serve directly as matmul stationary operands;
    biases ride on a bf16 ones-row K=1 matmul (all matmul inputs bf16 —
    the HW compiler rejects mixed 32/16-bit Matmult inputs).
  - per-sample attention runs with samples on partitions, entirely in bf16.
    BOTH directions are fused into each DVE/GPSIMD op with the direction
    index interleaved in the innermost stride:
      q [128,(h d i)]  k [128,(e d i)]  v [128,(d e i)]  out [128,(d h i)]
    so every operand view collapses to <=3 free dims (the HW TensorTensor
    limit) via (d i)/(e i) merges while staying innermost-packed for the
    DVE 2x perf mode — including the final tree levels, which fold the two
    directions' adjacent elements.
  - softmax weights are pre-normalized by 1/z at [128,128] instead of
    normalizing the [128,1024] output; reductions are in-place contiguous-
    slice binary trees inside the product buffers (no extra tiles).
  - work is split DVE/GPSIMD by head lanes (whole product+tree chains per
    lane) and software-pipelined over 5 stages per 128-sample tile:
    A proj+evac (PE/Act), B qk products, C qk tree+softmax+av products,
    D av tree, E transpose+staging+out-proj — each a tile behind the last,
    so cross-engine waits are covered by older tiles' ready work.
"""

import os

import numpy as np

import concourse.bacc as bacc
import concourse.mybir as mybir
import concourse.tile as tile
from concourse import bass_utils

B, NF = 32768, 512
H, DH = 8, 64
NCORES = 8
BC = B // NCORES          # samples per core
MT = 128                  # attention tile (samples)
ST = 512                  # projection super-tile (samples)
N_ST = BC // ST
N_MT = ST // MT
SCALE = 8.0               # sqrt(DH)

f32 = mybir.dt.float32
f32r = mybir.dt.float32r
bf16 = mybir.dt.bfloat16

WNAMES = ["q1", "k1", "v1", "q2", "k2", "v2"]

_CACHE: dict = {}


def _iget(name, default):
    return int(os.environ.get(name, str(default)))


def _emit(nc, tc, dr):
    from contextlib import ExitStack

    X = mybir.AxisListType.X
    ADD = mybir.AluOpType.add
    MULT = mybir.AluOpType.mult
    EXP = mybir.ActivationFunctionType.Exp
    IDENT_FN = mybir.ActivationFunctionType.Identity

    sc = _iget("BIDIR_SC_SPLIT", 6)    # qk-product head-lanes on DVE
    tq = _iget("BIDIR_TQ_SPLIT", 8)    # qk-tree head-lanes on DVE
    hv = _iget("BIDIR_AV_SPLIT", 6)    # av-product head-lanes on DVE
    tv = _iget("BIDIR_TV_SPLIT", 5)    # av-tree head-lanes on DVE
    lag = _iget("BIDIR_LAG", 1)
    lag2 = _iget("BIDIR_LAG2", 2)
    prenorm = _iget("BIDIR_PRENORM", 1)
    # the SWDGE accumulate-DMA path faults real HW (cost model liked it);
    # default OFF — the 64->32 level runs on the engine lanes instead
    qk_dma = _iget("BIDIR_QK_DMA", 0)

    # DVE: plain tensor_tensor hits the 2x bf16 perf mode; the
    # scalar_tensor_tensor form would drop to 1x (no perf modes) and the HW
    # compiler additionally limits it to 2-3D access patterns, so both
    # engines use plain TensorTensor for the broadcast-heavy attention ops.
    def vmul(out, a, b):
        nc.vector.tensor_mul(out, a, b)

    def vadd(out, a, b):
        nc.vector.tensor_add(out, a, b)

    def gmul(out, a, b):
        nc.gpsimd.tensor_mul(out, a, b)

    def gadd(out, a, b):
        nc.gpsimd.tensor_add(out, a, b)

    with ExitStack() as ctx:
        wpool = ctx.enter_context(tc.tile_pool(name="weights", bufs=1))
        xpool = ctx.enter_context(tc.tile_pool(name="xstage", bufs=_iget("BIDIR_X_BUFS", 2)))
        qkv_pool = ctx.enter_context(tc.tile_pool(name="qkv", bufs=_iget("BIDIR_QKV_BUFS", 3)))
        big_pool = ctx.enter_context(tc.tile_pool(name="bigp", bufs=_iget("BIDIR_BIG_BUFS", 3)))
        p2_pool = ctx.enter_context(tc.tile_pool(name="p2p", bufs=_iget("BIDIR_P2_BUFS", 2)))
        small_pool = ctx.enter_context(tc.tile_pool(name="small", bufs=_iget("BIDIR_SMALL_BUFS", 4)))
        ao_pool = ctx.enter_context(tc.tile_pool(name="aoT", bufs=_iget("BIDIR_AO_BUFS", 1)))
        ot_pool = ctx.enter_context(tc.tile_pool(name="otb", bufs=_iget("BIDIR_OT_BUFS", 2)))
        out_pool = ctx.enter_context(tc.tile_pool(name="outb", bufs=_iget("BIDIR_OUT_BUFS", 3)))
        pp = ctx.enter_context(tc.tile_pool(name="pproj", bufs=_iget("BIDIR_PP_BUFS", 1), space="PSUM"))
        pt = ctx.enter_context(tc.tile_pool(name="ptrans", bufs=1, space="PSUM"))
        po = ctx.enter_context(tc.tile_pool(name="pout", bufs=1, space="PSUM"))

        # ---- static weights ----
        W = {}
        for n in WNAMES:
            chunks = []
            for c in range(4):
                t = wpool.tile([128, NF], bf16, tag=f"w_{n}_{c}", name=f"w_{n}_{c}")
                nc.sync.dma_start(t[:], dr[f"w_{n}"].ap()[128 * c:128 * (c + 1), :])
                chunks.append(t)
            # bias row + ones stationary in bf16: halves the column footprint
            # and the bias contribution is tiny (0.02-scale), so bf16
            # rounding of it is harmless
            bt = wpool.tile([1, NF], bf16, tag=f"w_{n}_b", name=f"w_{n}_b")
            nc.sync.dma_start(bt[:], dr["wbias"].ap()[WNAMES.index(n):
                                                      WNAMES.index(n) + 1, :])
            W[n] = (chunks, bt[:])
        WO = {}
        for n in ("o1", "o2"):
            WO[n] = []
            for c in range(4):
                t = wpool.tile([128, NF], bf16, tag=f"wo_{n}_{c}", name=f"wo_{n}_{c}")
                nc.sync.dma_start(t[:], dr[f"w{n}T"].ap()[128 * c:128 * (c + 1), :])
                WO[n].append(t)
        bo_sb = {}
        for n in ("o1", "o2"):
            t = wpool.tile([128, 4], f32, tag=f"bo_{n}", name=f"bo_{n}")
            nc.sync.dma_start(t[:], dr[f"b{n}c"].ap())
            bo_sb[n] = t
        ones = wpool.tile([1, MT], bf16, tag="ones", name="ones")
        nc.sync.dma_start(ones[:], dr["ones_row"].ap())
        ident = wpool.tile([128, 128], bf16, tag="ident", name="ident")
        nc.sync.dma_start(ident[:], dr["ident"].ap())

        def load_supertile(s):
            ssl = slice(ST * s, ST * (s + 1))
            xs, ys = [], []
            for c in range(4):
                xt = xpool.tile([128, ST], bf16, tag=f"xs{c}", name=f"xs{c}")
                nc.sync.dma_start(xt[:], dr["xT"].ap()[128 * c:128 * (c + 1), ssl])
                xs.append(xt)
                yt = xpool.tile([128, ST], bf16, tag=f"ys{c}", name=f"ys{c}")
                nc.sync.dma_start(yt[:], dr["yT"].ap()[128 * c:128 * (c + 1), ssl])
                ys.append(yt)
            aoT = {1: ao_pool.tile([128, 4, ST], bf16, tag="aoT1", name="aoT1"),
                   2: ao_pool.tile([128, 4, ST], bf16, tag="aoT2", name="aoT2")}
            return {"xs": xs, "ys": ys, "aoT": aoT, "s": s, "done": 0}

        # The two directions are fused into single DVE/Pool ops with the
        # direction index i interleaved in the INNERMOST stride:
        #   q_sb [128, (h d i)]   k_sb [128, (e d i)]   v_sb [128, (d e i)]
        #   prod [128, m, (h e d2 i)]  p2 [128, (d h e i)]  o_t [128, (d h i)]
        # so every operand view collapses to <=3 free dims ((d i)/(e i)
        # merge), which the HW TensorTensor pattern requires, while staying
        # innermost-packed for the DVE 2x mode (incl. the final tree levels).

        def stage1a(sup, t):
            """Both directions' projections + evacs (PE/Act only).
            q,k of both directions are projected first so the DVE qk
            product can start before the v evacs land."""
            xs, ys = sup["xs"], sup["ys"]
            msl = slice(MT * t, MT * (t + 1))
            q_sb = qkv_pool.tile([128, NF, 2], bf16, tag="s_q", name="q_sb")
            k_sb = qkv_pool.tile([128, NF, 2], bf16, tag="s_k", name="k_sb")
            v_sb = qkv_pool.tile([128, NF, 2], bf16, tag="s_v", name="v_sb")
            plan = [("q1", 0, q_sb), ("k1", 0, k_sb),
                    ("q2", 1, q_sb), ("k2", 1, k_sb),
                    ("v1", 0, v_sb), ("v2", 1, v_sb)]
            for n, di, t_sb in plan:
                fr_x = (n[0] == "q") == (di == 0)
                srcn = xs if fr_x else ys
                psn = pp.tile([128, NF], f32, tag=f"p_{n[0]}{di}",
                              name=f"p_{n[0]}{di}")
                for c in range(4):
                    nc.tensor.matmul(psn[:], srcn[c][:, msl],
                                     W[n][0][c][:],
                                     start=(c == 0), stop=False)
                nc.tensor.matmul(psn[:], ones[:], W[n][1],
                                 start=False, stop=True)
                if n[0] == "q":
                    nc.scalar.mul(t_sb[:, :, di], psn[:], 1.0 / SCALE)
                elif n[0] == "k":
                    nc.scalar.copy(t_sb[:, :, di], psn[:])
                else:
                    # v stored d-major (d e i); psum is (e d)
                    nc.scalar.copy(
                        t_sb[:, :, di].rearrange("p (d e) -> p e d", e=H),
                        psn[:].rearrange("p (e d) -> p e d", e=H))
            return {"q_sb": q_sb, "k_sb": k_sb, "v_sb": v_sb,
                    "msl": msl, "sup": sup}

        def stage1b(st):
            """qk products (DVE head-lanes [0:sc), GPSIMD the rest).
            Written in two d-half blocks (m OUTERMOST) so the 64->32
            reduction is one contiguous accumulate-DMA."""
            q_sb, k_sb = st["q_sb"], st["k_sb"]
            W2 = DH  # merged (d_half i) width = 32*2
            prod = big_pool.tile([128, 2, H, H, W2], bf16,
                                 tag="bigbuf", name="qkprod")

            def qb_view(m, h0, h1):
                return (q_sb[:].rearrange("p (h m w) i -> p h (m w) i",
                                          h=H, m=2)
                        .rearrange("p h w i -> p h (w i)")
                        [:, h0:h1, W2 * m: W2 * (m + 1)]
                        .unsqueeze(2).broadcast_to([128, h1 - h0, H, W2]))

            def kb_view(m, h0, h1):
                return (k_sb[:].rearrange("p (e m w) i -> p e (m w) i",
                                          e=H, m=2)
                        .rearrange("p e w i -> p e (w i)")
                        [:, :, W2 * m: W2 * (m + 1)]
                        .unsqueeze(1).broadcast_to([128, h1 - h0, H, W2]))

            for m in (0, 1):
                if sc >= 8:
                    vmul(prod[:, m], qb_view(m, 0, 8), kb_view(m, 0, 8))
                else:
                    vmul(prod[:, m, 0:sc], qb_view(m, 0, sc),
                         kb_view(m, 0, sc))
                    gmul(prod[:, m, sc:8], qb_view(m, sc, 8),
                         kb_view(m, sc, 8))
            st["prod"] = prod

        def stage1b2(st):
            """64->32 qk reduction: contiguous accumulate-DMA (SWDGE), or
            DVE/Pool lanes when BIDIR_QK_DMA=0."""
            prod = st["prod"]
            if qk_dma:
                nc.gpsimd.dma_start(
                    prod[:, 0].rearrange("p h e w -> p (h e w)"),
                    prod[:, 1].rearrange("p h e w -> p (h e w)"),
                    accum_op=ADD)
            else:
                if tq >= 8:
                    vadd(prod[:, 0], prod[:, 0], prod[:, 1])
                else:
                    vadd(prod[:, 0, 0:tq], prod[:, 0, 0:tq],
                         prod[:, 1, 0:tq])
                    gadd(prod[:, 0, tq:8], prod[:, 0, tq:8],
                         prod[:, 1, tq:8])

        def stage1c(st):
            """qk tree -> softmax -> A@V products.  All tree levels are
            contiguous-slice in-place adds on the (d i)-merged axis, so
            even the last level stays innermost-packed (2x)."""
            prod, v_sb = st["prod"], st["v_sb"]
            tr = prod[:, 0]
            w2 = DH // 2
            while w2 >= 2:
                if tq >= 8:
                    vadd(tr[:, :, :, 0:w2], tr[:, :, :, 0:w2],
                         tr[:, :, :, w2:2 * w2])
                else:
                    vadd(tr[:, 0:tq, :, 0:w2], tr[:, 0:tq, :, 0:w2],
                         tr[:, 0:tq, :, w2:2 * w2])
                    gadd(tr[:, tq:8, :, 0:w2], tr[:, tq:8, :, 0:w2],
                         tr[:, tq:8, :, w2:2 * w2])
                w2 //= 2

            # softmax weights: A = exp(S) * (1/z)
            e_t = small_pool.tile([128, H, H, 2], bf16, tag="e_t", name="e_t")
            nc.scalar.activation(e_t[:], tr[:, :, :, 0:2], EXP)
            z_t = small_pool.tile([128, 2 * H], f32, tag="z_t", name="z_t")
            nc.vector.tensor_reduce(
                z_t[:].rearrange("p (h i) -> p h i", i=2),
                e_t[:].rearrange("p h e i -> p h i e"), axis=X, op=ADD)
            r_bf = small_pool.tile([128, 2 * H], bf16, tag="r_bf", name="r_bf")
            with nc.allow_low_precision(reason="1/z consumed in bf16 anyway"):
                nc.vector.reciprocal(r_bf[:], z_t[:])
            if prenorm:
                a_t = small_pool.tile([128, H, H, 2], bf16, tag="a_t",
                                      name="a_t")
                vmul(a_t[:], e_t[:],
                     r_bf[:].rearrange("p (h i) -> p h i", i=2)
                     .unsqueeze(2).broadcast_to([128, H, H, 2]))
            else:
                a_t = e_t

            # A@V product: p2 [128, (d h e i)], operands innermost-packed
            p2 = p2_pool.tile([128, DH, H, 2 * H], bf16, tag="p2buf",
                              name="avprod")
            a_b = (a_t[:].rearrange("p h e i -> p h (e i)")
                   .unsqueeze(1).broadcast_to([128, DH, H, 2 * H]))
            v_b = (v_sb[:].rearrange("p (d e) i -> p d (e i)", e=H)
                   .unsqueeze(2).broadcast_to([128, DH, H, 2 * H]))
            if hv >= 8:
                vmul(p2[:], a_b, v_b)
            elif hv <= 0:
                gmul(p2[:], a_b, v_b)
            else:
                vmul(p2[:, :, 0:hv], a_b[:, :, 0:hv], v_b[:, :, 0:hv])
                gmul(p2[:, :, hv:8], a_b[:, :, hv:8], v_b[:, :, hv:8])
            st["p2"] = p2
            st["r_bf"] = r_bf

        def stage2(st):
            """A@V e-reduction (contiguous in-place tree on the (e i) axis)
            -> o_t [128, (d h i)] (DVE/Pool)."""
            p2, msl, sup = st["p2"], st["msl"], st["sup"]
            for w2 in (8, 4):
                if tv >= 8:
                    vadd(p2[:, :, :, 0:w2], p2[:, :, :, 0:w2],
                         p2[:, :, :, w2:2 * w2])
                else:
                    vadd(p2[:, :, 0:tv, 0:w2], p2[:, :, 0:tv, 0:w2],
                         p2[:, :, 0:tv, w2:2 * w2])
                    gadd(p2[:, :, tv:8, 0:w2], p2[:, :, tv:8, 0:w2],
                         p2[:, :, tv:8, w2:2 * w2])
            o_t = ot_pool.tile([128, NF * 2], bf16, tag="o_t", name="o_t")
            o_v = o_t[:].rearrange("p (d h i) -> p d h i", h=H, i=2)
            if tv >= 8:
                vadd(o_v, p2[:, :, :, 0:2], p2[:, :, :, 2:4])
            else:
                vadd(o_v[:, :, 0:tv], p2[:, :, 0:tv, 0:2],
                     p2[:, :, 0:tv, 2:4])
                gadd(o_v[:, :, tv:8], p2[:, :, tv:8, 0:2],
                     p2[:, :, tv:8, 2:4])
            if not prenorm:
                o_n = ot_pool.tile([128, NF * 2], bf16, tag="o_n", name="o_n")
                vmul(o_n[:].rearrange("p (d h i) -> p d h i", h=H, i=2), o_v,
                     st["r_bf"][:].rearrange("p (h i) -> p h i", i=2)
                     .unsqueeze(1).broadcast_to([128, DH, H, 2]))
                o_t = o_n
            st["o_t"] = o_t

        def stage3(st):
            """transposes -> aoT staging (PE/Act only, lagged further so the
            slow DVE/Pool chain can't head-of-line-block projections);
            out-proj once a super-tile completes."""
            o_t, msl, sup = st["o_t"], st["msl"], st["sup"]
            o_4 = o_t[:].rearrange("p (d h i) -> p d h i", h=H, i=2)
            for di in range(2):
                tp = pt.tile([128, NF], bf16, tag="tp", name="tp")
                for c in range(4):
                    nc.tensor.transpose(
                        tp[:, 128 * c:128 * (c + 1)]
                        .rearrange("p (d h) -> p d h", h=H),
                        o_4[:, 16 * c:16 * (c + 1), :, di],
                        ident[:])
                nc.scalar.copy(sup["aoT"][di + 1][:, :, msl],
                               tp[:].rearrange("p (c m) -> p c m", c=4))
            sup["done"] += 1
            if sup["done"] == N_MT:
                out_projections(sup)

        def out_projections(sup):
            s, aoT = sup["s"], sup["aoT"]
            ssl = slice(ST * s, ST * (s + 1))
            for d, n in ((1, "o1"), (2, "o2")):
                od = dr["o1T"] if d == 1 else dr["o2T"]
                for o in range(4):
                    op_ps = po.tile([128, NF], f32, tag="op", name="op")
                    for c in range(4):
                        nc.tensor.matmul(op_ps[:],
                                         WO[n][c][:, 128 * o:128 * (o + 1)],
                                         aoT[d][:, c, :],
                                         start=(c == 0), stop=(c == 3))
                    ob = out_pool.tile([128, NF], f32, tag="ob", name="ob")
                    nc.scalar.activation(ob[:], op_ps[:], IDENT_FN,
                                         bias=bo_sb[n][:, o:o + 1], scale=1.0)
                    # store via the Act HWDGE queue: keeps the SP queue free
                    # for input loads, and the producer is Act anyway
                    nc.scalar.dma_start(od.ap()[128 * o:128 * (o + 1), ssl],
                                        ob[:])

        from collections import deque
        pipe0, pipe1, pipe2 = deque(), deque(), deque()

        order = _iget("BIDIR_ORDER", 1)

        def step(st=None):
            # 5-slot software pipeline, one tile per slot:
            #   A(t)   proj+evac            (PE/Act)
            #   B(t)   qk products          (DVE/Pool)
            #   C(t-1) qk l1+tree + softmax + av products
            #   D(t-2) av trees -> o_t      (DVE/Pool)
            #   E(t-3) transposes + staging (+ out-proj)
            # With BIDIR_ORDER=1 the older tiles' C/D blocks are emitted
            # BEFORE this tile's B products, so the DVE/Pool queues hold
            # ready work (t-1, t-2) in front of the products that must wait
            # for this tile's Act evacs.
            if st is not None and not order:
                stage1b(st)
                pipe0.append(st)
            if len(pipe0) > lag or (st is None and pipe0):
                stC = pipe0.popleft()
                stage1b2(stC)
                stage1c(stC)
                pipe1.append(stC)
            if len(pipe1) > lag or (st is None and pipe1):
                stD = pipe1.popleft()
                stage2(stD)
                pipe2.append(stD)
            if st is not None and order:
                stage1b(st)
                pipe0.append(st)
            if len(pipe2) > lag2 or (st is None and pipe2):
                stage3(pipe2.popleft())

        for s in range(N_ST):
            sup = load_supertile(s)
            for t in range(N_MT):
                step(stage1a(sup, t))
        while pipe0 or pipe1 or pipe2:
            step()


def _get_module():
    if "nc" in _CACHE:
        return _CACHE["nc"]
    nc = bacc.Bacc("TRN2", target_bir_lowering=False, debug=False,
                   enable_asserts=True, num_devices=NCORES)
    dr = {}
    dr["xT"] = nc.dram_tensor("xT", [NF, BC], bf16, kind="ExternalInput")
    dr["yT"] = nc.dram_tensor("yT", [NF, BC], bf16, kind="ExternalInput")
    for n in WNAMES:
        dr[f"w_{n}"] = nc.dram_tensor(f"w_{n}", [NF, NF], bf16,
                                      kind="ExternalInput")
    dr["wbias"] = nc.dram_tensor("wbias", [len(WNAMES), NF], bf16,
                                 kind="ExternalInput")
    dr["wo1T"] = nc.dram_tensor("wo1T", [NF, NF], bf16, kind="ExternalInput")
    dr["wo2T"] = nc.dram_tensor("wo2T", [NF, NF], bf16, kind="ExternalInput")
    dr["bo1c"] = nc.dram_tensor("bo1c", [128, 4], f32, kind="ExternalInput")
    dr["bo2c"] = nc.dram_tensor("bo2c", [128, 4], f32, kind="ExternalInput")
    dr["ones_row"] = nc.dram_tensor("ones_row", [1, MT], bf16,
                                    kind="ExternalInput")
    dr["ident"] = nc.dram_tensor("ident", [128, 128], bf16, kind="ExternalInput")
    dr["o1T"] = nc.dram_tensor("o1T", [NF, BC], f32, kind="ExternalOutput")
    dr["o2T"] = nc.dram_tensor("o2T", [NF, BC], f32, kind="ExternalOutput")

    with tile.TileContext(nc) as tc:
        _emit(nc, tc, dr)
    nc.compile()
    _CACHE["nc"] = nc
    return nc


def _prepare_in_maps(inputs):
    import ml_dtypes

    prot = np.asarray(inputs["protein_features"], dtype=np.float32)
    lig = np.asarray(inputs["ligand_features"], dtype=np.float32)

    shared = {}
    for n in WNAMES:
        wt = np.asarray(inputs[f"W{n}"], dtype=np.float32).T
        shared[f"w_{n}"] = np.ascontiguousarray(wt).astype(ml_dtypes.bfloat16)
    shared["wbias"] = np.stack(
        [np.asarray(inputs[f"b{n}"], dtype=np.float32) for n in WNAMES]
    ).astype(ml_dtypes.bfloat16)
    # attention output is d-major (feature d*8+h); permute Wo rows to match
    idx = np.arange(NF)
    perm = (idx % H) * DH + (idx // H)   # dest row d*8+h <- src row h*64+d
    shared["wo1T"] = np.ascontiguousarray(
        np.asarray(inputs["Wo1"], dtype=np.float32).T[perm]).astype(
        ml_dtypes.bfloat16)
    shared["wo2T"] = np.ascontiguousarray(
        np.asarray(inputs["Wo2"], dtype=np.float32).T[perm]).astype(
        ml_dtypes.bfloat16)
    shared["bo1c"] = np.ascontiguousarray(
        np.asarray(inputs["bo1"], dtype=np.float32).reshape(4, 128).T)
    shared["bo2c"] = np.ascontiguousarray(
        np.asarray(inputs["bo2"], dtype=np.float32).reshape(4, 128).T)
    shared["ones_row"] = np.ones((1, MT), dtype=ml_dtypes.bfloat16)
    shared["ident"] = np.eye(128, dtype=ml_dtypes.bfloat16)

    in_maps = []
    for c in range(NCORES):
        sl = slice(c * BC, (c + 1) * BC)
        m = dict(shared)
        m["xT"] = np.ascontiguousarray(prot[sl].T).astype(ml_dtypes.bfloat16)
        m["yT"] = np.ascontiguousarray(lig[sl].T).astype(ml_dtypes.bfloat16)
        in_maps.append(m)
    return in_maps


def _run(inputs, trace=False, tmpdir=None):
    nc = _get_module()
    in_maps = _prepare_in_maps(inputs)
    res = bass_utils.run_bass_kernel_spmd(
        nc, in_maps, core_ids=list(range(NCORES)), trace=trace, tmpdir=tmpdir)

    p_out = np.empty((B, NF), dtype=np.float32)
    l_out = np.empty((B, NF), dtype=np.float32)
    for c in range(NCORES):
        sl = slice(c * BC, (c + 1) * BC)
        p_out[sl] = res.results[c]["o1T"].T
        l_out[sl] = res.results[c]["o2T"].T
    return (p_out, l_out), res


def kernel(**inputs):
    out, _ = _run(inputs, trace=bool(int(os.environ.get("BIDIR_TRACE", "0"))))
    return out

